# revision 31
# baseline (speedup 1.0000x reference)
"""Trainium2 Bass kernel for nn_Encoding_layer (highway stack + pairwise MLP
attention + fuse gates).

Sharding: data-parallel over batch B=16 across 8 NeuronCores (2 batches per
core); all dense weights replicated. No collectives.

Mixed fp8/bf16 design:
  - Attention scores + numerator/denominator and the fuse z-gate run in
    fp8e4 with perf_mode=DoubleRow (two 128-contraction k-tiles per
    instruction).  Highway layer 0 is also fp8-DoubleRow and interleaved
    into the input-transpose phase so the PE chews matmuls while the
    remaining input tiles DMA in.  fp8 weights are scaled x16 before the
    cast (raw 0.02-scale weights sit in e4m3's subnormal range); the 1/16
    is folded into the scalar-engine activation `scale` on the psum drain.
  - Highway layer 1 and the fuse r-gate's inputs-half stay bf16: their
    error reaches the output un-smoothed (r multiplies raw inputs), while
    the fp8 parts only reach it through the near-uniform softmax over
    L=1024 (32x noise dilution) or the saturating z-gate.
  - gpsimd never touches fp8 or PSUM (ucode fallback measured ~36us/op).

Per-core layouts (n = 2 batches x L=1024 = 2048 token-columns):
  xT8 (fp8) / xTb (bf16)   : [128, 4, 2048] inputs^T  [u mod 128, u div 128, n]
  x1Tb bf16, x2T8/w3x8/attT8 fp8 : same layout
  xO8                      : [128, 16, 512] fp8 row-major highway output
  Attention: S^T[j,i] = s3[j,i] (PE, w3*x^T as lhsT) + s2[j] (ACT exp bias).
  The per-column term s1[i]+ab never enters the matmuls: a per-column
  factor cancels in the softmax, so relu becomes a clamp against
  th[i] = exp(-(s1[i]+ab)).  Numerator att^T (lhsT = row-major x,
  DoubleRow over j-tile pairs) and denominator (lhsT = fp8 ones
  [128,2,16] -> 16 identical psum rows) come from matmuls against M^T;
  normalization multiplies by the broadcast fast-approx reciprocal.
  Broadcasts of [1,512] rows are PE outer-products + scalar copies.
"""

import numpy as np

B, L, U, H = 16, 1024, 512, 2
NCORES = 8
BPC = B // NCORES          # batches per core
N = BPC * L                # token columns per core
KU = U // 128              # 4  u-tiles
NT = N // 128              # 16 row-tiles per core
NS = N // 512              # 4  512-wide column slices per core
JT = L // 128              # 8  j-tiles per batch
IH = L // 512              # 2  i-halves per batch
WS = 16.0                  # fp8 weight scale
IWS = 1.0 / WS


def build_nc():
    import concourse.bacc as bacc
    import concourse.tile as tile
    from concourse import mybir
    from concourse.masks import make_identity

    F32 = mybir.dt.float32
    BF16 = mybir.dt.bfloat16
    FP8 = mybir.dt.float8e4
    AF = mybir.ActivationFunctionType
    OP = mybir.AluOpType
    DR = mybir.MatmulPerfMode.DoubleRow

    nc = bacc.Bacc("TRN2", target_bir_lowering=False, debug=False,
                   num_devices=NCORES)

    x_in = nc.dram_tensor("inputs", [BPC, L, U], F32, kind="ExternalInput").ap()
    tW = nc.dram_tensor("tW", [H, U, U], F32, kind="ExternalInput").ap()
    tb = nc.dram_tensor("tb", [H, U], F32, kind="ExternalInput").ap()
    cW = nc.dram_tensor("cW", [H, U, U], F32, kind="ExternalInput").ap()
    cb = nc.dram_tensor("cb", [H, U], F32, kind="ExternalInput").ap()
    aW = nc.dram_tensor("aW", [3 * U], F32, kind="ExternalInput").ap()
    ab = nc.dram_tensor("ab", [1], F32, kind="ExternalInput").ap()
    frW = nc.dram_tensor("frW", [2 * U, U], F32, kind="ExternalInput").ap()
    frb = nc.dram_tensor("frb", [U], F32, kind="ExternalInput").ap()
    ffW = nc.dram_tensor("ffW", [2 * U, U], F32, kind="ExternalInput").ap()
    ffb = nc.dram_tensor("ffb", [U], F32, kind="ExternalInput").ap()
    out = nc.dram_tensor("out", [BPC, L, U], F32, kind="ExternalOutput").ap()

    xv = x_in.flatten_outer_dims().rearrange("(t p) u -> t p u", p=128)
    outv = out.flatten_outer_dims().rearrange("(t p) u -> t p u", p=128)

    fWv = ffW.rearrange("(k p) m -> k p m", p=128)
    rWv = frW.rearrange("(k p) m -> k p m", p=128)

    with tile.TileContext(nc) as tc:
        with tc.tile_pool(name="pers", bufs=1) as pers:
            # ---- persistent SBUF tensors ----
            xT8 = pers.tile([128, KU, N], FP8, tag="xT8")     # inputs^T fp8
            xTb = pers.tile([128, KU, N], BF16, tag="xTb")    # inputs^T bf16
            x1Tb = pers.tile([128, KU, N], BF16, tag="x1Tb")
            x2T8 = pers.tile([128, KU, N], FP8, tag="x2T8")
            w3x8 = pers.tile([128, KU, N], FP8, tag="w3x8")
            attT8 = pers.tile([128, KU, N], FP8, tag="attT8")
            xO8 = pers.tile([128, NT, U], FP8, tag="xO8")
            tW8 = pers.tile([128, KU, U], FP8, tag="tW8")     # layer0, x16
            cW8 = pers.tile([128, KU, U], FP8, tag="cW8")     # layer0, x16
            tWb = pers.tile([128, KU, U], BF16, tag="tWb")    # layer1
            cWb = pers.tile([128, KU, U], BF16, tag="cWb")    # layer1
            fW8 = pers.tile([128, 2 * KU, U], FP8, tag="fW8")  # x16
            rWb16 = pers.tile([128, KU, U], BF16, tag="rWb16")  # x16
            rW8 = pers.tile([128, KU, U], FP8, tag="rW8")       # x16
            tbsb = pers.tile([128, H, KU], F32, tag="tbsb")
            cbsb = pers.tile([128, H, KU], F32, tag="cbsb")
            awsb = pers.tile([128, 12], F32, tag="awsb")      # w1|w2|w3 cols
            aw3 = pers.tile([128, KU], F32, tag="aw3")        # 16*w3
            w1h8p = pers.tile([128, KU, 16], FP8, tag="w1h8p")  # x16, col0
            w2h8 = pers.tile([128, KU, 1], FP8, tag="w2h8")   # x16
            ab_sb = pers.tile([1, 1], F32, tag="ab_sb")
            nab_sb = pers.tile([1, 1], F32, tag="nab_sb")
            ffb16 = pers.tile([1, U], BF16, tag="ffb16")      # x16
            frb16 = pers.tile([1, U], BF16, tag="frb16")      # x16
            thr = pers.tile([1, N], BF16, tag="thr")   # exp(-(s1+ab))
            s2f = pers.tile([128, NT], F32, tag="s2f")
            ones_row = pers.tile([1, 128], BF16, tag="ones_row")
            ones216 = pers.tile([128, 2, 16], FP8, tag="ones216")
            identf = pers.tile([128, 128], F32, tag="identf")
            ident8 = pers.tile([128, 128], FP8, tag="ident8")

            nc.vector.memset(ones_row, 1.0)
            nc.vector.memset(ones216, 1.0)
            make_identity(nc, identf)
            make_identity(nc, ident8)

            # fuse-gate weight chunks dripped through phases B+C and D
            fuse_chunks = (
                [(fWv, fW8, k, k, FP8) for k in range(2 * KU)] +
                [(rWv, rWb16, k, k, BF16) for k in range(KU)] +
                [(rWv, rW8, k, k - KU, FP8) for k in range(KU, 2 * KU)])

            def emit_fuse(ci):
                wv_, wdst_, ksrc_, kdst_, dt_ = fuse_chunks[ci]
                wsf = pers.tile([128, U], F32, tag="wsf", bufs=4,
                                name=f"wsf_{ci}")
                nc.sync.dma_start(wsf, wv_[ksrc_])
                if ci % 2 == 0:
                    nc.vector.tensor_scalar_mul(wdst_[:, kdst_, :], wsf, WS)
                else:
                    nc.scalar.mul(wdst_[:, kdst_, :], wsf, WS)

            # ======== Phase A: loads, transpose, highway layer 0 ==========
            with tc.tile_pool(name="stg", bufs=8) as stg, \
                 tc.tile_pool(name="stgw", bufs=8) as stgw, \
                 tc.tile_pool(name="stgf", bufs=4) as stgf, \
                 tc.tile_pool(name="warmP", bufs=1, space="PSUM") as warmP, \
                 tc.tile_pool(name="transP", bufs=1, space="PSUM") as transP, \
                 tc.tile_pool(name="hw0P", bufs=3, space="PSUM") as hw0P:
                def emit_weights(l, wi):
                    wsrc = (tW, cW)[wi]
                    wdst = ((tW8, cW8), (tWb, cWb))[l][wi]
                    wv = wsrc[l].rearrange("(k p) m -> k p m", p=128)
                    for k in range(KU):
                        ws = stgw.tile([128, U], F32, tag="ws",
                                       name=f"ws_{l}_{wi}_{k}")
                        nc.sync.dma_start(ws, wv[k])
                        if l == 0:
                            if k % 2 == 0:
                                nc.vector.tensor_scalar_mul(
                                    wdst[:, k, :], ws, WS)
                            else:
                                nc.scalar.mul(wdst[:, k, :], ws, WS)
                        else:
                            if k % 2 == 0:
                                nc.vector.tensor_copy(wdst[:, k, :], ws)
                            else:
                                nc.scalar.copy(wdst[:, k, :], ws)

                # small tensors first (layer-0 needs biases)
                nc.sync.dma_start(
                    tbsb, tb.rearrange("l (m p) -> p l m", p=128))
                nc.sync.dma_start(
                    cbsb, cb.rearrange("l (m p) -> p l m", p=128))
                nc.sync.dma_start(
                    awsb, aW.rearrange("(w m p) -> p (w m)", p=128, w=3))
                nc.vector.memset(w1h8p, 0.0)
                nc.vector.tensor_scalar_mul(w1h8p[:, :, 0], awsb[:, 0:KU],
                                            WS)
                nc.vector.tensor_scalar_mul(
                    w2h8[:, :, 0], awsb[:, KU:2 * KU], WS)
                nc.vector.tensor_scalar_mul(aw3, awsb[:, 2 * KU:3 * KU], WS)
                nc.sync.dma_start(ab_sb, ab[None, :])
                nc.scalar.mul(nab_sb, ab_sb, -1.0)
                fb = stg.tile([1, U], F32, tag="fb")
                nc.sync.dma_start(fb, ffb[None, :])
                nc.scalar.mul(ffb16, fb, WS)
                fb2 = stg.tile([1, U], F32, tag="fb")
                nc.sync.dma_start(fb2, frb[None, :])
                nc.scalar.mul(frb16, fb2, WS)

                # warm the PE HAM clock-gate during the initial DMA wait
                warm = warmP.tile([128, 512], F32, tag="warm0",
                                  name="warm_0")
                for i in range(24):
                    nc.tensor.matmul(warm[:, 0:128], identf, identf,
                                     start=True, stop=True)

                def trans_block(tg, tt):
                    t = tg * 4 + tt
                    xs = stg.tile([128, U], F32, tag="xs",
                                  name=f"xs_{t}")
                    nc.sync.dma_start(xs, xv[t])
                    ptt = transP.tile([128, 512], F32, tag="ptt",
                                      name=f"ptt_{t}")
                    for k in range(KU):
                        nc.tensor.transpose(
                            ptt[:, k * 128:(k + 1) * 128],
                            xs[:, k * 128:(k + 1) * 128], identf)
                    csl = slice(tg * 512 + tt * 128,
                                tg * 512 + (tt + 1) * 128)
                    pv = ptt.rearrange("p (k c) -> p k c", k=KU)
                    nc.vector.tensor_copy(xT8[:, :, csl], pv)
                    if tt % 2 == 0:
                        nc.scalar.copy(xTb[:, :, csl], pv)
                    else:
                        nc.vector.tensor_copy(xTb[:, :, csl], pv)

                for tt in range(4):
                    trans_block(0, tt)
                emit_weights(0, 0)
                emit_weights(0, 1)
                def trans_block(tg, tt):
                    t = tg * 4 + tt
                    xs = stg.tile([128, U], F32, tag="xs",
                                  name=f"xs_{t}")
                    nc.sync.dma_start(xs, xv[t])
                    ptt = transP.tile([128, 512], F32, tag="ptt",
                                      name=f"ptt_{t}")
                    for k in range(KU):
                        nc.tensor.transpose(
                            ptt[:, k * 128:(k + 1) * 128],
                            xs[:, k * 128:(k + 1) * 128], identf)
                    csl = slice(tg * 512 + tt * 128,
                                tg * 512 + (tt + 1) * 128)
                    pv = ptt.rearrange("p (k c) -> p k c", k=KU)
                    nc.vector.tensor_copy(xT8[:, :, csl], pv)
                    if tt % 2 == 0:
                        nc.scalar.copy(xTb[:, :, csl], pv)
                    else:
                        nc.vector.tensor_copy(xTb[:, :, csl], pv)

                for tt in range(4):
                    trans_block(0, tt)
                emit_weights(0, 0)
                emit_weights(0, 1)
                for tg in range(NS):
                    nsl = slice(tg * 512, (tg + 1) * 512)
                    # highway layer 0 (fp8 DR), with tg+1's transposes
                    # interleaved between DR groups to keep HAM warm and
                    # hide the transpose latency
                    for m in range(KU):
                        pt = hw0P.tile([128, 512], F32, tag="pt")
                        pc = hw0P.tile([128, 512], F32, tag="pc")
                        for kk in range(2):
                            ksl = slice(2 * kk, 2 * kk + 2)
                            nc.tensor.matmul(
                                pt, tW8[:, ksl, m * 128:(m + 1) * 128],
                                xT8[:, ksl, nsl],
                                start=(kk == 0), stop=(kk == 1),
                                perf_mode=DR)
                        for kk in range(2):
                            ksl = slice(2 * kk, 2 * kk + 2)
                            nc.tensor.matmul(
                                pc, cW8[:, ksl, m * 128:(m + 1) * 128],
                                xT8[:, ksl, nsl],
                                start=(kk == 0), stop=(kk == 1),
                                perf_mode=DR)
                        if tg + 1 < NS:
                            trans_block(tg + 1, m)
                        th = stg.tile([128, 512], BF16, tag="th",
                                      name=f"th0_{tg}_{m}")
                        ch = stg.tile([128, 512], BF16, tag="ch",
                                      name=f"ch0_{tg}_{m}")
                        nc.scalar.activation(
                            th, pt, AF.Relu, bias=tbsb[:, 0, m:m + 1],
                            scale=IWS)
                        nc.scalar.activation(
                            ch, pc, AF.Sigmoid, bias=cbsb[:, 0, m:m + 1],
                            scale=IWS)
                        dh = stg.tile([128, 512], BF16, tag="dh",
                                      name=f"dh0_{tg}_{m}")
                        nc.vector.tensor_tensor(
                            dh, th, xTb[:, m, nsl], op=OP.subtract)
                        mh = stg.tile([128, 512], BF16, tag="mh",
                                      name=f"mh0_{tg}_{m}")
                        nc.gpsimd.tensor_tensor(mh, ch, dh, op=OP.mult)
                        nc.vector.tensor_tensor(
                            x1Tb[:, m, nsl], xTb[:, m, nsl], mh, op=OP.add)
                    # drip layer-1 weights behind the layer-0 compute
                    if tg == 0:
                        emit_weights(1, 0)
                    elif tg == 1:
                        emit_weights(1, 1)

            # ===== Phase B+C: highway layer 1 (bf16) fused with
                # attention prep (xO8 transposes, w3x, s1, s2) ============
                with tc.tile_pool(name="hwp", bufs=2, space="PSUM") as hwp, \
                     tc.tile_pool(name="xop", bufs=2, space="PSUM") as xop, \
                     tc.tile_pool(name="pc1", bufs=1, space="PSUM") as pc1, \
                     tc.tile_pool(name="hws", bufs=3) as hws:
                    ps1 = pc1.tile([16, 512], F32, tag="ps1")
                s2p = pc1.tile([128, NT], F32, tag="s2p")

                def l1_block(t):
                    nsl = slice(t * 512, (t + 1) * 512)
                    for m in range(KU):
                        pt = hwp.tile([128, 512], F32, tag="pt",
                                      name=f"pt1_{t}_{m}")
                        pc = hwp.tile([128, 512], F32, tag="pc",
                                      name=f"pc1_{t}_{m}")
                        for k in range(KU):
                            nc.tensor.matmul(
                                pt, tWb[:, k, m * 128:(m + 1) * 128],
                                x1Tb[:, k, nsl],
                                start=(k == 0), stop=(k == KU - 1))
                        for k in range(KU):
                            nc.tensor.matmul(
                                pc, cWb[:, k, m * 128:(m + 1) * 128],
                                x1Tb[:, k, nsl],
                                start=(k == 0), stop=(k == KU - 1))
                        th = hws.tile([128, 512], BF16, tag="th",
                                      name=f"th1_{t}_{m}")
                        ch = hws.tile([128, 512], BF16, tag="ch",
                                      name=f"ch1_{t}_{m}")
                        nc.scalar.activation(
                            th, pt, AF.Relu, bias=tbsb[:, 1, m:m + 1])
                        nc.scalar.activation(
                            ch, pc, AF.Sigmoid, bias=cbsb[:, 1, m:m + 1])
                        dh = hws.tile([128, 512], BF16, tag="dh",
                                      name=f"dh1_{t}_{m}")
                        nc.vector.tensor_tensor(
                            dh, th, x1Tb[:, m, nsl], op=OP.subtract)
                        mh = hws.tile([128, 512], BF16, tag="mh",
                                      name=f"mh1_{t}_{m}")
                        nc.gpsimd.tensor_tensor(mh, ch, dh, op=OP.mult)
                        nc.vector.tensor_tensor(
                            x2T8[:, m, nsl], x1Tb[:, m, nsl], mh,
                            op=OP.add)

                def prep_block(t):
                    # attention prep for a finished 512-column group;
                    # emitted one slice late so the in-order PE queue
                    # never waits on the x2 elementwise chain
                    nsl = slice(t * 512, (t + 1) * 512)
                    for k in range(KU):
                        nc.vector.tensor_scalar_mul(
                            w3x8[:, k, nsl], x2T8[:, k, nsl],
                            aw3[:, k:k + 1])
                    for jt in range(4 * t, 4 * t + 4):
                        ptr = xop.tile([128, 512, 2], FP8, tag="ptr",
                                       name=f"ptr_{jt}")
                        for k in range(KU):
                            nc.tensor.transpose(
                                ptr[:, k * 128:(k + 1) * 128, 0],
                                x2T8[:, k, jt * 128:(jt + 1) * 128],
                                ident8)
                        if jt % 2 == 0:
                            nc.vector.tensor_copy(
                                xO8[:, jt, :], ptr[:, :, 0])
                        else:
                            nc.scalar.copy(xO8[:, jt, :], ptr[:, :, 0])
                        for k in range(KU):
                            nc.tensor.matmul(
                                s2p[:, jt:jt + 1],
                                x2T8[:, k, jt * 128:(jt + 1) * 128],
                                w2h8[:, k, :],
                                start=(k == 0), stop=(k == KU - 1))
                    for kk in range(2):
                        ksl = slice(2 * kk, 2 * kk + 2)
                        nc.tensor.matmul(ps1, w1h8p[:, ksl, :],
                                         x2T8[:, ksl, nsl],
                                         start=(kk == 0), stop=(kk == 1),
                                         perf_mode=DR)
                    nc.scalar.activation(
                        thr[:, nsl], ps1[0:1, :], AF.Exp,
                        bias=nab_sb, scale=-IWS)
                    emit_fuse(2 * t)
                    emit_fuse(2 * t + 1)

                for t in range(NS):
                    l1_block(t)
                    if t > 0:
                        prep_block(t - 1)
                prep_block(NS - 1)
                nc.scalar.mul(s2f, s2p, IWS)

            # ============= Phase D: pairwise softmax attention =============
            with tc.tile_pool(name="pdn", bufs=1, space="PSUM") as pdn, \
                 tc.tile_pool(name="pds", bufs=3, space="PSUM") as pds, \
                 tc.tile_pool(name="prp", bufs=1, space="PSUM") as prp, \
                 tc.tile_pool(name="stgf", bufs=4) as stgf, \
                 tc.tile_pool(name="dsb", bufs=4) as dsb:
                def make_tail(b, h, isl, rec, pn, pnh):
                    def tail():
                        rech = dsb.tile([1, 512], BF16, tag="rech",
                                        name=f"rech_{b}_{h}")
                        nc.vector.tensor_copy(rech, rec)
                        rbc = dsb.tile([128, 512], BF16, tag="rbc",
                                       name=f"rbc_{b}_{h}")
                        pb2 = pds.tile([128, 512], F32, tag="ps",
                                       name=f"pb2_{b}_{h}")
                        nc.tensor.matmul(pb2, ones_row, rech,
                                         start=True, stop=True)
                        nc.scalar.copy(rbc, pb2)
                        for du in range(KU):
                            nc.vector.tensor_tensor(
                                attT8[:, du, isl], pnh[du], rbc,
                                op=OP.mult)
                    return tail

                deferred = None
                for b in range(BPC):
                    for h in range(IH):
                        unit = b * IH + h
                        # remaining fuse-gate weight chunks (2 per unit)
                        if unit < 4:
                            emit_fuse(8 + 2 * unit)
                            emit_fuse(8 + 2 * unit + 1)
                        isl = slice(b * L + h * 512, b * L + (h + 1) * 512)
                        pn = [pdn.tile([128, 512], F32, tag=f"pn{du}",
                                       name=f"pn_{b}_{h}_{du}")
                              for du in range(KU)]
                        pr16 = prp.tile([16, 512], F32, tag="pr16",
                                        name=f"pr16_{b}_{h}")
                        thbc = dsb.tile([128, 512], BF16, tag="thbc")
                        pb1 = pds.tile([128, 512], F32, tag="ps",
                                       name=f"pb1_{b}_{h}")
                        nc.tensor.matmul(pb1, ones_row, thr[:, isl],
                                         start=True, stop=True)
                        nc.scalar.copy(thbc, pb1)
                        for jp in range(JT // 2):
                            eh2 = dsb.tile([128, 2, 512], FP8, tag="eh2",
                                           name=f"eh2_{b}_{h}_{jp}")
                            eh2b = dsb.tile([128, 2, 512], BF16, tag="eh2b",
                                            name=f"eh2b_{b}_{h}_{jp}")
                            for g in range(2):
                                jt = 2 * jp + g
                                jg = b * JT + jt
                                jsl = slice(b * L + jt * 128,
                                            b * L + (jt + 1) * 128)
                                ps = pds.tile([128, 512], F32, tag="ps",
                                              name=f"ps_{b}_{h}_{jt}")
                                for kk in range(2):
                                    ksl = slice(2 * kk, 2 * kk + 2)
                                    nc.tensor.matmul(
                                        ps, w3x8[:, ksl, jsl],
                                        x2T8[:, ksl, isl],
                                        start=(kk == 0), stop=(kk == 1),
                                        perf_mode=DR)
                                nc.scalar.activation(
                                    eh2b[:, g, :], ps, AF.Exp,
                                    bias=s2f[:, jg:jg + 1], scale=IWS)
                                nc.vector.tensor_tensor(
                                    eh2[:, g, :], eh2b[:, g, :], thbc,
                                    op=OP.max)
                            jg0 = b * JT + 2 * jp
                            for du in range(KU):
                                nc.tensor.matmul(
                                    pn[du],
                                    xO8[:, jg0:jg0 + 2,
                                        du * 128:(du + 1) * 128],
                                    eh2,
                                    start=(jp == 0), stop=(jp == 3),
                                    perf_mode=DR)
                            nc.tensor.matmul(pr16, ones216, eh2,
                                             start=(jp == 0), stop=(jp == 3),
                                             perf_mode=DR)
                        rec = dsb.tile([1, 512], F32, tag="rec",
                                       name=f"rec_{b}_{h}")
                        nc.vector.reciprocal_approx_fast(rec, pr16[0:1, :])
                        # drain pn psum banks promptly (frees them for the
                        # next unit); the normalize tail is deferred past
                        # the next unit's matmuls so the in-order PE queue
                        # never waits on the reciprocal chain
                        pnh = [dsb.tile([128, 512], BF16, tag="pnh",
                                        bufs=8, name=f"pnh_{b}_{h}_{du}")
                               for du in range(KU)]
                        for du in range(KU):
                            if du % 2 == 0:
                                nc.scalar.copy(pnh[du], pn[du])
                            else:
                                nc.vector.tensor_copy(pnh[du], pn[du])
                        if deferred is not None:
                            deferred()
                        deferred = make_tail(b, h, isl, rec, pn, pnh)
                deferred()

            # ============= Phase E: fuse gates + output ====================
            with tc.tile_pool(name="pep", bufs=3, space="PSUM") as pep, \
                 tc.tile_pool(name="esb", bufs=3) as esb:
                for mt in range(NT):
                    msl = slice(mt * 128, (mt + 1) * 128)
                    x0t = esb.tile([128, U], F32, tag="x0t")
                    nc.sync.dma_start(x0t, xv[mt])
                    pz = pep.tile([128, 512], F32, tag="pz")
                    pr2 = pep.tile([128, 512], F32, tag="pr2")
                    for kk in range(4):
                        if kk < 2:
                            lhsT = xT8[:, 2 * kk:2 * kk + 2, msl]
                        else:
                            lhsT = attT8[:, 2 * (kk - 2):2 * (kk - 2) + 2,
                                         msl]
                        wsl = slice(2 * kk, 2 * kk + 2)
                        nc.tensor.matmul(pz, lhsT, fW8[:, wsl, :],
                                         start=(kk == 0), stop=False,
                                         perf_mode=DR)
                    for k in range(KU):
                        nc.tensor.matmul(pr2, xTb[:, k, msl], rWb16[:, k, :],
                                         start=(k == 0), stop=False)
                    for kk in range(2):
                        ksl = slice(2 * kk, 2 * kk + 2)
                        nc.tensor.matmul(pr2, attT8[:, ksl, msl],
                                         rW8[:, ksl, :],
                                         start=False, stop=False,
                                         perf_mode=DR)
                    nc.tensor.matmul(pz, ones_row, ffb16, start=False,
                                     stop=True)
                    nc.tensor.matmul(pr2, ones_row, frb16, start=False,
                                     stop=True)
                    zh = esb.tile([128, U], BF16, tag="zh")
                    rh = esb.tile([128, U], BF16, tag="rh")
                    q = esb.tile([128, U], BF16, tag="q")
                    p2 = esb.tile([128, U], F32, tag="p2")
                    ot = esb.tile([128, U], F32, tag="ot")
                    if mt == NT - 1:
                        # last unit sets the kernel tail: shorten its
                        # serial chain by splitting across engines
                        hU = U // 2
                        nc.scalar.activation(zh, pz, AF.Sigmoid, scale=IWS)
                        nc.vector.tensor_tensor(q, zh, zh, op=OP.mult)
                        nc.scalar.activation(rh, pr2, AF.Sigmoid, scale=IWS)
                        nc.vector.tensor_tensor(p2[:, :hU], rh[:, :hU],
                                                x0t[:, :hU], op=OP.mult)
                        nc.gpsimd.tensor_tensor(p2[:, hU:], rh[:, hU:],
                                                x0t[:, hU:], op=OP.mult)
                        nc.vector.tensor_tensor(ot[:, :hU], q[:, :hU],
                                                p2[:, :hU], op=OP.add)
                        nc.gpsimd.tensor_tensor(ot[:, hU:], q[:, hU:],
                                                p2[:, hU:], op=OP.add)
                    else:
                        nc.scalar.activation(zh, pz, AF.Sigmoid, scale=IWS)
                        nc.scalar.activation(rh, pr2, AF.Sigmoid, scale=IWS)
                        nc.vector.tensor_tensor(q, zh, zh, op=OP.mult)
                        nc.gpsimd.tensor_tensor(p2, rh, x0t, op=OP.mult)
                        nc.vector.tensor_tensor(ot, q, p2, op=OP.add)
                    nc.sync.dma_start(outv[mt], ot)

    nc.compile()
    return nc


_NC_CACHE = None


def _get_nc():
    global _NC_CACHE
    if _NC_CACHE is None:
        _NC_CACHE = build_nc()
    return _NC_CACHE


def kernel(**inputs) -> np.ndarray:
    from concourse.bass_utils import run_bass_kernel_spmd

    nc = _get_nc()
    full = {k: np.ascontiguousarray(np.asarray(v, dtype=np.float32))
            for k, v in inputs.items()}
    in_maps = []
    for c in range(NCORES):
        m = dict(full)
        m["inputs"] = np.ascontiguousarray(
            full["inputs"][c * BPC:(c + 1) * BPC])
        in_maps.append(m)
    res = run_bass_kernel_spmd(nc, in_maps, core_ids=list(range(NCORES)))
    return np.concatenate([res.results[c]["out"] for c in range(NCORES)],
                          axis=0)


# revision 32
# speedup vs baseline: 1.0399x; 1.0399x over previous
"""Trainium2 Bass kernel for nn_Encoding_layer (highway stack + pairwise MLP
attention + fuse gates).

Sharding: data-parallel over batch B=16 across 8 NeuronCores (2 batches per
core); all dense weights replicated. No collectives.

Mixed fp8/bf16 design:
  - Attention scores + numerator/denominator and the fuse z-gate run in
    fp8e4 with perf_mode=DoubleRow (two 128-contraction k-tiles per
    instruction).  Highway layer 0 is also fp8-DoubleRow and interleaved
    into the input-transpose phase so the PE chews matmuls while the
    remaining input tiles DMA in.  fp8 weights are scaled x16 before the
    cast (raw 0.02-scale weights sit in e4m3's subnormal range); the 1/16
    is folded into the scalar-engine activation `scale` on the psum drain.
  - Highway layer 1 and the fuse r-gate's inputs-half stay bf16: their
    error reaches the output un-smoothed (r multiplies raw inputs), while
    the fp8 parts only reach it through the near-uniform softmax over
    L=1024 (32x noise dilution) or the saturating z-gate.
  - gpsimd never touches fp8 or PSUM (ucode fallback measured ~36us/op).

Per-core layouts (n = 2 batches x L=1024 = 2048 token-columns):
  xT8 (fp8) / xTb (bf16)   : [128, 4, 2048] inputs^T  [u mod 128, u div 128, n]
  x1Tb bf16, x2T8/w3x8/attT8 fp8 : same layout
  xO8                      : [128, 16, 512] fp8 row-major highway output
  Attention: S^T[j,i] = s3[j,i] (PE, w3*x^T as lhsT) + s2[j] (ACT exp bias).
  The per-column term s1[i]+ab never enters the matmuls: a per-column
  factor cancels in the softmax, so relu becomes a clamp against
  th[i] = exp(-(s1[i]+ab)).  Numerator att^T (lhsT = row-major x,
  DoubleRow over j-tile pairs) and denominator (lhsT = fp8 ones
  [128,2,16] -> 16 identical psum rows) come from matmuls against M^T;
  normalization multiplies by the broadcast fast-approx reciprocal.
  Broadcasts of [1,512] rows are PE outer-products + scalar copies.
"""

import numpy as np

B, L, U, H = 16, 1024, 512, 2
NCORES = 8
BPC = B // NCORES          # batches per core
N = BPC * L                # token columns per core
KU = U // 128              # 4  u-tiles
NT = N // 128              # 16 row-tiles per core
NS = N // 512              # 4  512-wide column slices per core
JT = L // 128              # 8  j-tiles per batch
IH = L // 512              # 2  i-halves per batch
WS = 16.0                  # fp8 weight scale
IWS = 1.0 / WS


def build_nc():
    import concourse.bacc as bacc
    import concourse.tile as tile
    from concourse import mybir
    from concourse.masks import make_identity

    F32 = mybir.dt.float32
    BF16 = mybir.dt.bfloat16
    FP8 = mybir.dt.float8e4
    AF = mybir.ActivationFunctionType
    OP = mybir.AluOpType
    DR = mybir.MatmulPerfMode.DoubleRow

    nc = bacc.Bacc("TRN2", target_bir_lowering=False, debug=False,
                   num_devices=NCORES)

    x_in = nc.dram_tensor("inputs", [BPC, L, U], F32, kind="ExternalInput").ap()
    tW = nc.dram_tensor("tW", [H, U, U], F32, kind="ExternalInput").ap()
    tb = nc.dram_tensor("tb", [H, U], F32, kind="ExternalInput").ap()
    cW = nc.dram_tensor("cW", [H, U, U], F32, kind="ExternalInput").ap()
    cb = nc.dram_tensor("cb", [H, U], F32, kind="ExternalInput").ap()
    aW = nc.dram_tensor("aW", [3 * U], F32, kind="ExternalInput").ap()
    ab = nc.dram_tensor("ab", [1], F32, kind="ExternalInput").ap()
    frW = nc.dram_tensor("frW", [2 * U, U], F32, kind="ExternalInput").ap()
    frb = nc.dram_tensor("frb", [U], F32, kind="ExternalInput").ap()
    ffW = nc.dram_tensor("ffW", [2 * U, U], F32, kind="ExternalInput").ap()
    ffb = nc.dram_tensor("ffb", [U], F32, kind="ExternalInput").ap()
    out = nc.dram_tensor("out", [BPC, L, U], F32, kind="ExternalOutput").ap()

    xv = x_in.flatten_outer_dims().rearrange("(t p) u -> t p u", p=128)
    outv = out.flatten_outer_dims().rearrange("(t p) u -> t p u", p=128)

    fWv = ffW.rearrange("(k p) m -> k p m", p=128)
    rWv = frW.rearrange("(k p) m -> k p m", p=128)

    with tile.TileContext(nc) as tc:
        with tc.tile_pool(name="pers", bufs=1) as pers:
            # ---- persistent SBUF tensors ----
            xT8 = pers.tile([128, KU, N], FP8, tag="xT8")     # inputs^T fp8
            xTb = pers.tile([128, KU, N], BF16, tag="xTb")    # inputs^T bf16
            x1Tb = pers.tile([128, KU, N], BF16, tag="x1Tb")
            x2T8 = pers.tile([128, KU, N], FP8, tag="x2T8")
            w3x8 = pers.tile([128, KU, N], FP8, tag="w3x8")
            attT8 = pers.tile([128, KU, N], FP8, tag="attT8")
            xO8 = pers.tile([128, NT, U], FP8, tag="xO8")
            tW8 = pers.tile([128, KU, U], FP8, tag="tW8")     # layer0, x16
            cW8 = pers.tile([128, KU, U], FP8, tag="cW8")     # layer0, x16
            tWb = pers.tile([128, KU, U], BF16, tag="tWb")    # layer1
            cWb = pers.tile([128, KU, U], BF16, tag="cWb")    # layer1
            fW8 = pers.tile([128, 2 * KU, U], FP8, tag="fW8")  # x16
            rWb16 = pers.tile([128, KU, U], BF16, tag="rWb16")  # x16
            rW8 = pers.tile([128, KU, U], FP8, tag="rW8")       # x16
            tbsb = pers.tile([128, H, KU], F32, tag="tbsb")
            cbsb = pers.tile([128, H, KU], F32, tag="cbsb")
            awsb = pers.tile([128, 12], F32, tag="awsb")      # w1|w2|w3 cols
            aw3 = pers.tile([128, KU], F32, tag="aw3")        # 16*w3
            w1h8p = pers.tile([128, KU, 16], FP8, tag="w1h8p")  # x16, col0
            w2h8 = pers.tile([128, KU, 1], FP8, tag="w2h8")   # x16
            ab_sb = pers.tile([1, 1], F32, tag="ab_sb")
            nab_sb = pers.tile([1, 1], F32, tag="nab_sb")
            ffb16 = pers.tile([1, U], BF16, tag="ffb16")      # x16
            frb16 = pers.tile([1, U], BF16, tag="frb16")      # x16
            thr = pers.tile([1, N], BF16, tag="thr")   # exp(-(s1+ab))
            s2f = pers.tile([128, NT], F32, tag="s2f")
            ones_row = pers.tile([1, 128], BF16, tag="ones_row")
            ones216 = pers.tile([128, 2, 16], FP8, tag="ones216")
            identf = pers.tile([128, 128], F32, tag="identf")
            ident8 = pers.tile([128, 128], FP8, tag="ident8")

            nc.vector.memset(ones_row, 1.0)
            nc.vector.memset(ones216, 1.0)
            make_identity(nc, identf)
            make_identity(nc, ident8)

            # fuse-gate weight chunks dripped through phases B+C and D
            fuse_chunks = (
                [(fWv, fW8, k, k, FP8) for k in range(2 * KU)] +
                [(rWv, rWb16, k, k, BF16) for k in range(KU)] +
                [(rWv, rW8, k, k - KU, FP8) for k in range(KU, 2 * KU)])

            def emit_fuse(ci):
                wv_, wdst_, ksrc_, kdst_, dt_ = fuse_chunks[ci]
                wsf = pers.tile([128, U], F32, tag="wsf", bufs=4,
                                name=f"wsf_{ci}")
                nc.sync.dma_start(wsf, wv_[ksrc_])
                if ci % 2 == 0:
                    nc.vector.tensor_scalar_mul(wdst_[:, kdst_, :], wsf, WS)
                else:
                    nc.scalar.mul(wdst_[:, kdst_, :], wsf, WS)

            # ======== Phase A: loads, transpose, highway layer 0 ==========
            with tc.tile_pool(name="stg", bufs=8) as stg, \
                 tc.tile_pool(name="stgw", bufs=8) as stgw, \
                 tc.tile_pool(name="stgf", bufs=4) as stgf, \
                 tc.tile_pool(name="warmP", bufs=1, space="PSUM") as warmP, \
                 tc.tile_pool(name="transP", bufs=2, space="PSUM") as transP, \
                 tc.tile_pool(name="hw0P", bufs=2, space="PSUM") as hw0P:
                def emit_weights(l, wi):
                    wsrc = (tW, cW)[wi]
                    wdst = ((tW8, cW8), (tWb, cWb))[l][wi]
                    wv = wsrc[l].rearrange("(k p) m -> k p m", p=128)
                    for k in range(KU):
                        ws = stgw.tile([128, U], F32, tag="ws",
                                       name=f"ws_{l}_{wi}_{k}")
                        nc.sync.dma_start(ws, wv[k])
                        if l == 0:
                            if k % 2 == 0:
                                nc.vector.tensor_scalar_mul(
                                    wdst[:, k, :], ws, WS)
                            else:
                                nc.scalar.mul(wdst[:, k, :], ws, WS)
                        else:
                            if k % 2 == 0:
                                nc.vector.tensor_copy(wdst[:, k, :], ws)
                            else:
                                nc.scalar.copy(wdst[:, k, :], ws)

                # small tensors first (layer-0 needs biases)
                nc.sync.dma_start(
                    tbsb, tb.rearrange("l (m p) -> p l m", p=128))
                nc.sync.dma_start(
                    cbsb, cb.rearrange("l (m p) -> p l m", p=128))
                nc.sync.dma_start(
                    awsb, aW.rearrange("(w m p) -> p (w m)", p=128, w=3))
                nc.vector.memset(w1h8p, 0.0)
                nc.vector.tensor_scalar_mul(w1h8p[:, :, 0], awsb[:, 0:KU],
                                            WS)
                nc.vector.tensor_scalar_mul(
                    w2h8[:, :, 0], awsb[:, KU:2 * KU], WS)
                nc.vector.tensor_scalar_mul(aw3, awsb[:, 2 * KU:3 * KU], WS)
                nc.sync.dma_start(ab_sb, ab[None, :])
                nc.scalar.mul(nab_sb, ab_sb, -1.0)
                fb = stg.tile([1, U], F32, tag="fb")
                nc.sync.dma_start(fb, ffb[None, :])
                nc.scalar.mul(ffb16, fb, WS)
                fb2 = stg.tile([1, U], F32, tag="fb")
                nc.sync.dma_start(fb2, frb[None, :])
                nc.scalar.mul(frb16, fb2, WS)

                # warm the PE HAM clock-gate during the initial DMA wait
                warm = [warmP.tile([128, 512], F32, tag=f"warm{k}",
                                   name=f"warm_{k}") for k in range(2)]
                for i in range(24):
                    nc.tensor.matmul(warm[i % 2][:, 0:128], identf, identf,
                                     start=True, stop=True)

                def trans_block(tg, tt):
                    t = tg * 4 + tt
                    xs = stg.tile([128, U], F32, tag="xs",
                                  name=f"xs_{t}")
                    nc.sync.dma_start(xs, xv[t])
                    ptt = transP.tile([128, 512], F32, tag="ptt",
                                      name=f"ptt_{t}")
                    for k in range(KU):
                        nc.tensor.transpose(
                            ptt[:, k * 128:(k + 1) * 128],
                            xs[:, k * 128:(k + 1) * 128], identf)
                    csl = slice(tg * 512 + tt * 128,
                                tg * 512 + (tt + 1) * 128)
                    pv = ptt.rearrange("p (k c) -> p k c", k=KU)
                    nc.vector.tensor_copy(xT8[:, :, csl], pv)
                    if tt % 2 == 0:
                        nc.scalar.copy(xTb[:, :, csl], pv)
                    else:
                        nc.vector.tensor_copy(xTb[:, :, csl], pv)

                for tt in range(4):
                    trans_block(0, tt)
                emit_weights(0, 0)
                emit_weights(0, 1)
                def trans_block(tg, tt):
                    t = tg * 4 + tt
                    xs = stg.tile([128, U], F32, tag="xs",
                                  name=f"xs_{t}")
                    nc.sync.dma_start(xs, xv[t])
                    ptt = transP.tile([128, 512], F32, tag="ptt",
                                      name=f"ptt_{t}")
                    for k in range(KU):
                        nc.tensor.transpose(
                            ptt[:, k * 128:(k + 1) * 128],
                            xs[:, k * 128:(k + 1) * 128], identf)
                    csl = slice(tg * 512 + tt * 128,
                                tg * 512 + (tt + 1) * 128)
                    pv = ptt.rearrange("p (k c) -> p k c", k=KU)
                    nc.vector.tensor_copy(xT8[:, :, csl], pv)
                    if tt % 2 == 0:
                        nc.scalar.copy(xTb[:, :, csl], pv)
                    else:
                        nc.vector.tensor_copy(xTb[:, :, csl], pv)

                for tt in range(4):
                    trans_block(0, tt)
                emit_weights(0, 0)
                emit_weights(0, 1)
                for tg in range(NS):
                    nsl = slice(tg * 512, (tg + 1) * 512)
                    # highway layer 0 (fp8 DR), with tg+1's transposes
                    # interleaved between DR groups to keep HAM warm and
                    # hide the transpose latency
                    for m in range(KU):
                        pt = hw0P.tile([128, 512], F32, tag="pt")
                        pc = hw0P.tile([128, 512], F32, tag="pc")
                        for kk in range(2):
                            ksl = slice(2 * kk, 2 * kk + 2)
                            nc.tensor.matmul(
                                pt, tW8[:, ksl, m * 128:(m + 1) * 128],
                                xT8[:, ksl, nsl],
                                start=(kk == 0), stop=(kk == 1),
                                perf_mode=DR)
                        for kk in range(2):
                            ksl = slice(2 * kk, 2 * kk + 2)
                            nc.tensor.matmul(
                                pc, cW8[:, ksl, m * 128:(m + 1) * 128],
                                xT8[:, ksl, nsl],
                                start=(kk == 0), stop=(kk == 1),
                                perf_mode=DR)
                        if tg + 1 < NS:
                            trans_block(tg + 1, m)
                        th = stg.tile([128, 512], BF16, tag="th",
                                      name=f"th0_{tg}_{m}")
                        ch = stg.tile([128, 512], BF16, tag="ch",
                                      name=f"ch0_{tg}_{m}")
                        nc.scalar.activation(
                            th, pt, AF.Relu, bias=tbsb[:, 0, m:m + 1],
                            scale=IWS)
                        nc.scalar.activation(
                            ch, pc, AF.Sigmoid, bias=cbsb[:, 0, m:m + 1],
                            scale=IWS)
                        dh = stg.tile([128, 512], BF16, tag="dh",
                                      name=f"dh0_{tg}_{m}")
                        nc.vector.tensor_tensor(
                            dh, th, xTb[:, m, nsl], op=OP.subtract)
                        mh = stg.tile([128, 512], BF16, tag="mh",
                                      name=f"mh0_{tg}_{m}")
                        nc.gpsimd.tensor_tensor(mh, ch, dh, op=OP.mult)
                        nc.vector.tensor_tensor(
                            x1Tb[:, m, nsl], xTb[:, m, nsl], mh, op=OP.add)
                    # drip layer-1 weights behind the layer-0 compute
                    if tg == 0:
                        emit_weights(1, 0)
                    elif tg == 1:
                        emit_weights(1, 1)

            # ===== Phase B+C: highway layer 1 (bf16) fused with
                # attention prep (xO8 transposes, w3x, s1, s2) ============
                with tc.tile_pool(name="hwp", bufs=2, space="PSUM") as hwp, \
                     tc.tile_pool(name="xop", bufs=2, space="PSUM") as xop, \
                     tc.tile_pool(name="pc1", bufs=1, space="PSUM") as pc1, \
                     tc.tile_pool(name="hws", bufs=3) as hws:
                    ps1 = pc1.tile([16, 512], F32, tag="ps1")
                s2p = pc1.tile([128, NT], F32, tag="s2p")

                def l1_block(t):
                    nsl = slice(t * 512, (t + 1) * 512)
                    for m in range(KU):
                        pt = hwp.tile([128, 512], F32, tag="pt",
                                      name=f"pt1_{t}_{m}")
                        pc = hwp.tile([128, 512], F32, tag="pc",
                                      name=f"pc1_{t}_{m}")
                        for k in range(KU):
                            nc.tensor.matmul(
                                pt, tWb[:, k, m * 128:(m + 1) * 128],
                                x1Tb[:, k, nsl],
                                start=(k == 0), stop=(k == KU - 1))
                        for k in range(KU):
                            nc.tensor.matmul(
                                pc, cWb[:, k, m * 128:(m + 1) * 128],
                                x1Tb[:, k, nsl],
                                start=(k == 0), stop=(k == KU - 1))
                        th = hws.tile([128, 512], BF16, tag="th",
                                      name=f"th1_{t}_{m}")
                        ch = hws.tile([128, 512], BF16, tag="ch",
                                      name=f"ch1_{t}_{m}")
                        nc.scalar.activation(
                            th, pt, AF.Relu, bias=tbsb[:, 1, m:m + 1])
                        nc.scalar.activation(
                            ch, pc, AF.Sigmoid, bias=cbsb[:, 1, m:m + 1])
                        dh = hws.tile([128, 512], BF16, tag="dh",
                                      name=f"dh1_{t}_{m}")
                        nc.vector.tensor_tensor(
                            dh, th, x1Tb[:, m, nsl], op=OP.subtract)
                        mh = hws.tile([128, 512], BF16, tag="mh",
                                      name=f"mh1_{t}_{m}")
                        nc.gpsimd.tensor_tensor(mh, ch, dh, op=OP.mult)
                        nc.vector.tensor_tensor(
                            x2T8[:, m, nsl], x1Tb[:, m, nsl], mh,
                            op=OP.add)

                def prep_block(t):
                    # attention prep for a finished 512-column group;
                    # emitted one slice late so the in-order PE queue
                    # never waits on the x2 elementwise chain
                    nsl = slice(t * 512, (t + 1) * 512)
                    for k in range(KU):
                        nc.vector.tensor_scalar_mul(
                            w3x8[:, k, nsl], x2T8[:, k, nsl],
                            aw3[:, k:k + 1])
                    for jt in range(4 * t, 4 * t + 4):
                        ptr = xop.tile([128, 512, 2], FP8, tag="ptr",
                                       name=f"ptr_{jt}")
                        for k in range(KU):
                            nc.tensor.transpose(
                                ptr[:, k * 128:(k + 1) * 128, 0],
                                x2T8[:, k, jt * 128:(jt + 1) * 128],
                                ident8)
                        if jt % 2 == 0:
                            nc.vector.tensor_copy(
                                xO8[:, jt, :], ptr[:, :, 0])
                        else:
                            nc.scalar.copy(xO8[:, jt, :], ptr[:, :, 0])
                        for k in range(KU):
                            nc.tensor.matmul(
                                s2p[:, jt:jt + 1],
                                x2T8[:, k, jt * 128:(jt + 1) * 128],
                                w2h8[:, k, :],
                                start=(k == 0), stop=(k == KU - 1))
                    for kk in range(2):
                        ksl = slice(2 * kk, 2 * kk + 2)
                        nc.tensor.matmul(ps1, w1h8p[:, ksl, :],
                                         x2T8[:, ksl, nsl],
                                         start=(kk == 0), stop=(kk == 1),
                                         perf_mode=DR)
                    nc.scalar.activation(
                        thr[:, nsl], ps1[0:1, :], AF.Exp,
                        bias=nab_sb, scale=-IWS)
                    emit_fuse(2 * t)
                    emit_fuse(2 * t + 1)

                for t in range(NS):
                    l1_block(t)
                    if t > 0:
                        prep_block(t - 1)
                prep_block(NS - 1)
                nc.scalar.mul(s2f, s2p, IWS)

            # ============= Phase D: pairwise softmax attention =============
            with tc.tile_pool(name="pdn", bufs=1, space="PSUM") as pdn, \
                 tc.tile_pool(name="pds", bufs=3, space="PSUM") as pds, \
                 tc.tile_pool(name="prp", bufs=1, space="PSUM") as prp, \
                 tc.tile_pool(name="stgf", bufs=4) as stgf, \
                 tc.tile_pool(name="dsb", bufs=4) as dsb:
                def make_tail(b, h, isl, rec, pn, pnh):
                    def tail():
                        rech = dsb.tile([1, 512], BF16, tag="rech",
                                        name=f"rech_{b}_{h}")
                        nc.vector.tensor_copy(rech, rec)
                        rbc = dsb.tile([128, 512], BF16, tag="rbc",
                                       name=f"rbc_{b}_{h}")
                        pb2 = pds.tile([128, 512], F32, tag="ps",
                                       name=f"pb2_{b}_{h}")
                        nc.tensor.matmul(pb2, ones_row, rech,
                                         start=True, stop=True)
                        nc.scalar.copy(rbc, pb2)
                        for du in range(KU):
                            nc.vector.tensor_tensor(
                                attT8[:, du, isl], pnh[du], rbc,
                                op=OP.mult)
                    return tail

                deferred = None
                for b in range(BPC):
                    for h in range(IH):
                        unit = b * IH + h
                        # remaining fuse-gate weight chunks (2 per unit)
                        if unit < 4:
                            emit_fuse(8 + 2 * unit)
                            emit_fuse(8 + 2 * unit + 1)
                        isl = slice(b * L + h * 512, b * L + (h + 1) * 512)
                        pn = [pdn.tile([128, 512], F32, tag=f"pn{du}",
                                       name=f"pn_{b}_{h}_{du}")
                              for du in range(KU)]
                        pr16 = prp.tile([16, 512], F32, tag="pr16",
                                        name=f"pr16_{b}_{h}")
                        thbc = dsb.tile([128, 512], BF16, tag="thbc")
                        pb1 = pds.tile([128, 512], F32, tag="ps",
                                       name=f"pb1_{b}_{h}")
                        nc.tensor.matmul(pb1, ones_row, thr[:, isl],
                                         start=True, stop=True)
                        nc.scalar.copy(thbc, pb1)
                        for jp in range(JT // 2):
                            eh2 = dsb.tile([128, 2, 512], FP8, tag="eh2",
                                           name=f"eh2_{b}_{h}_{jp}")
                            eh2b = dsb.tile([128, 2, 512], BF16, tag="eh2b",
                                            name=f"eh2b_{b}_{h}_{jp}")
                            for g in range(2):
                                jt = 2 * jp + g
                                jg = b * JT + jt
                                jsl = slice(b * L + jt * 128,
                                            b * L + (jt + 1) * 128)
                                ps = pds.tile([128, 512], F32, tag="ps",
                                              name=f"ps_{b}_{h}_{jt}")
                                for kk in range(2):
                                    ksl = slice(2 * kk, 2 * kk + 2)
                                    nc.tensor.matmul(
                                        ps, w3x8[:, ksl, jsl],
                                        x2T8[:, ksl, isl],
                                        start=(kk == 0), stop=(kk == 1),
                                        perf_mode=DR)
                                nc.scalar.activation(
                                    eh2b[:, g, :], ps, AF.Exp,
                                    bias=s2f[:, jg:jg + 1], scale=IWS)
                                nc.vector.tensor_tensor(
                                    eh2[:, g, :], eh2b[:, g, :], thbc,
                                    op=OP.max)
                            jg0 = b * JT + 2 * jp
                            for du in range(KU):
                                nc.tensor.matmul(
                                    pn[du],
                                    xO8[:, jg0:jg0 + 2,
                                        du * 128:(du + 1) * 128],
                                    eh2,
                                    start=(jp == 0), stop=(jp == 3),
                                    perf_mode=DR)
                            nc.tensor.matmul(pr16, ones216, eh2,
                                             start=(jp == 0), stop=(jp == 3),
                                             perf_mode=DR)
                        rec = dsb.tile([1, 512], F32, tag="rec",
                                       name=f"rec_{b}_{h}")
                        nc.vector.reciprocal_approx_fast(rec, pr16[0:1, :])
                        # drain pn psum banks promptly (frees them for the
                        # next unit); the normalize tail is deferred past
                        # the next unit's matmuls so the in-order PE queue
                        # never waits on the reciprocal chain
                        pnh = [dsb.tile([128, 512], BF16, tag="pnh",
                                        bufs=8, name=f"pnh_{b}_{h}_{du}")
                               for du in range(KU)]
                        for du in range(KU):
                            if du % 2 == 0:
                                nc.scalar.copy(pnh[du], pn[du])
                            else:
                                nc.vector.tensor_copy(pnh[du], pn[du])
                        if deferred is not None:
                            deferred()
                        deferred = make_tail(b, h, isl, rec, pn, pnh)
                deferred()

            # ============= Phase E: fuse gates + output ====================
            with tc.tile_pool(name="pep", bufs=2, space="PSUM") as pep, \
                 tc.tile_pool(name="esb", bufs=3) as esb:
                for mt in range(NT):
                    msl = slice(mt * 128, (mt + 1) * 128)
                    x0t = esb.tile([128, U], F32, tag="x0t")
                    nc.sync.dma_start(x0t, xv[mt])
                    pz = pep.tile([128, 512], F32, tag="pz")
                    pr2 = pep.tile([128, 512], F32, tag="pr2")
                    for kk in range(4):
                        if kk < 2:
                            lhsT = xT8[:, 2 * kk:2 * kk + 2, msl]
                        else:
                            lhsT = attT8[:, 2 * (kk - 2):2 * (kk - 2) + 2,
                                         msl]
                        wsl = slice(2 * kk, 2 * kk + 2)
                        nc.tensor.matmul(pz, lhsT, fW8[:, wsl, :],
                                         start=(kk == 0), stop=False,
                                         perf_mode=DR)
                    for k in range(KU):
                        nc.tensor.matmul(pr2, xTb[:, k, msl], rWb16[:, k, :],
                                         start=(k == 0), stop=False)
                    for kk in range(2):
                        ksl = slice(2 * kk, 2 * kk + 2)
                        nc.tensor.matmul(pr2, attT8[:, ksl, msl],
                                         rW8[:, ksl, :],
                                         start=False, stop=False,
                                         perf_mode=DR)
                    nc.tensor.matmul(pz, ones_row, ffb16, start=False,
                                     stop=True)
                    nc.tensor.matmul(pr2, ones_row, frb16, start=False,
                                     stop=True)
                    zh = esb.tile([128, U], BF16, tag="zh")
                    rh = esb.tile([128, U], BF16, tag="rh")
                    q = esb.tile([128, U], BF16, tag="q")
                    p2 = esb.tile([128, U], F32, tag="p2")
                    ot = esb.tile([128, U], F32, tag="ot")
                    if mt == NT - 1:
                        # last unit sets the kernel tail: shorten its
                        # serial chain by splitting across engines
                        hU = U // 2
                        nc.scalar.activation(zh, pz, AF.Sigmoid, scale=IWS)
                        nc.vector.tensor_tensor(q, zh, zh, op=OP.mult)
                        nc.scalar.activation(rh, pr2, AF.Sigmoid, scale=IWS)
                        nc.vector.tensor_tensor(p2[:, :hU], rh[:, :hU],
                                                x0t[:, :hU], op=OP.mult)
                        nc.gpsimd.tensor_tensor(p2[:, hU:], rh[:, hU:],
                                                x0t[:, hU:], op=OP.mult)
                        nc.vector.tensor_tensor(ot[:, :hU], q[:, :hU],
                                                p2[:, :hU], op=OP.add)
                        nc.gpsimd.tensor_tensor(ot[:, hU:], q[:, hU:],
                                                p2[:, hU:], op=OP.add)
                    else:
                        nc.scalar.activation(zh, pz, AF.Sigmoid, scale=IWS)
                        nc.scalar.activation(rh, pr2, AF.Sigmoid, scale=IWS)
                        nc.vector.tensor_tensor(q, zh, zh, op=OP.mult)
                        nc.gpsimd.tensor_tensor(p2, rh, x0t, op=OP.mult)
                        nc.vector.tensor_tensor(ot, q, p2, op=OP.add)
                    nc.sync.dma_start(outv[mt], ot)

    nc.compile()
    return nc


_NC_CACHE = None


def _get_nc():
    global _NC_CACHE
    if _NC_CACHE is None:
        _NC_CACHE = build_nc()
    return _NC_CACHE


def kernel(**inputs) -> np.ndarray:
    from concourse.bass_utils import run_bass_kernel_spmd

    nc = _get_nc()
    full = {k: np.ascontiguousarray(np.asarray(v, dtype=np.float32))
            for k, v in inputs.items()}
    in_maps = []
    for c in range(NCORES):
        m = dict(full)
        m["inputs"] = np.ascontiguousarray(
            full["inputs"][c * BPC:(c + 1) * BPC])
        in_maps.append(m)
    res = run_bass_kernel_spmd(nc, in_maps, core_ids=list(range(NCORES)))
    return np.concatenate([res.results[c]["out"] for c in range(NCORES)],
                          axis=0)


# revision 33
# speedup vs baseline: 1.0477x; 1.0075x over previous
"""Trainium2 Bass kernel for nn_Encoding_layer (highway stack + pairwise MLP
attention + fuse gates).

Sharding: data-parallel over batch B=16 across 8 NeuronCores (2 batches per
core); all dense weights replicated. No collectives.

Mixed fp8/bf16 design:
  - Attention scores + numerator/denominator and the fuse z-gate run in
    fp8e4 with perf_mode=DoubleRow (two 128-contraction k-tiles per
    instruction).  Highway layer 0 is also fp8-DoubleRow and interleaved
    into the input-transpose phase so the PE chews matmuls while the
    remaining input tiles DMA in.  fp8 weights are scaled x16 before the
    cast (raw 0.02-scale weights sit in e4m3's subnormal range); the 1/16
    is folded into the scalar-engine activation `scale` on the psum drain.
  - Highway layer 1 and the fuse r-gate's inputs-half stay bf16: their
    error reaches the output un-smoothed (r multiplies raw inputs), while
    the fp8 parts only reach it through the near-uniform softmax over
    L=1024 (32x noise dilution) or the saturating z-gate.
  - gpsimd never touches fp8 or PSUM (ucode fallback measured ~36us/op).

Per-core layouts (n = 2 batches x L=1024 = 2048 token-columns):
  xT8 (fp8) / xTb (bf16)   : [128, 4, 2048] inputs^T  [u mod 128, u div 128, n]
  x1Tb bf16, x2T8/w3x8/attT8 fp8 : same layout
  xO8                      : [128, 16, 512] fp8 row-major highway output
  Attention: S^T[j,i] = s3[j,i] (PE, w3*x^T as lhsT) + s2[j] (ACT exp bias).
  The per-column term s1[i]+ab never enters the matmuls: a per-column
  factor cancels in the softmax, so relu becomes a clamp against
  th[i] = exp(-(s1[i]+ab)).  Numerator att^T (lhsT = row-major x,
  DoubleRow over j-tile pairs) and denominator (lhsT = fp8 ones
  [128,2,16] -> 16 identical psum rows) come from matmuls against M^T;
  normalization multiplies by the broadcast fast-approx reciprocal.
  Broadcasts of [1,512] rows are PE outer-products + scalar copies.
"""

import numpy as np

B, L, U, H = 16, 1024, 512, 2
NCORES = 8
BPC = B // NCORES          # batches per core
N = BPC * L                # token columns per core
KU = U // 128              # 4  u-tiles
NT = N // 128              # 16 row-tiles per core
NS = N // 512              # 4  512-wide column slices per core
JT = L // 128              # 8  j-tiles per batch
IH = L // 512              # 2  i-halves per batch
WS = 16.0                  # fp8 weight scale
IWS = 1.0 / WS


def build_nc():
    import concourse.bacc as bacc
    import concourse.tile as tile
    from concourse import mybir
    from concourse.masks import make_identity

    F32 = mybir.dt.float32
    BF16 = mybir.dt.bfloat16
    FP8 = mybir.dt.float8e4
    AF = mybir.ActivationFunctionType
    OP = mybir.AluOpType
    DR = mybir.MatmulPerfMode.DoubleRow

    nc = bacc.Bacc("TRN2", target_bir_lowering=False, debug=False,
                   num_devices=NCORES)

    x_in = nc.dram_tensor("inputs", [BPC, L, U], F32, kind="ExternalInput").ap()
    tW = nc.dram_tensor("tW", [H, U, U], F32, kind="ExternalInput").ap()
    tb = nc.dram_tensor("tb", [H, U], F32, kind="ExternalInput").ap()
    cW = nc.dram_tensor("cW", [H, U, U], F32, kind="ExternalInput").ap()
    cb = nc.dram_tensor("cb", [H, U], F32, kind="ExternalInput").ap()
    aW = nc.dram_tensor("aW", [3 * U], F32, kind="ExternalInput").ap()
    ab = nc.dram_tensor("ab", [1], F32, kind="ExternalInput").ap()
    frW = nc.dram_tensor("frW", [2 * U, U], F32, kind="ExternalInput").ap()
    frb = nc.dram_tensor("frb", [U], F32, kind="ExternalInput").ap()
    ffW = nc.dram_tensor("ffW", [2 * U, U], F32, kind="ExternalInput").ap()
    ffb = nc.dram_tensor("ffb", [U], F32, kind="ExternalInput").ap()
    out = nc.dram_tensor("out", [BPC, L, U], F32, kind="ExternalOutput").ap()

    xv = x_in.flatten_outer_dims().rearrange("(t p) u -> t p u", p=128)
    outv = out.flatten_outer_dims().rearrange("(t p) u -> t p u", p=128)

    fWv = ffW.rearrange("(k p) m -> k p m", p=128)
    rWv = frW.rearrange("(k p) m -> k p m", p=128)

    with tile.TileContext(nc) as tc:
        with tc.tile_pool(name="pers", bufs=1) as pers:
            # ---- persistent SBUF tensors ----
            xT8 = pers.tile([128, KU, N], FP8, tag="xT8")     # inputs^T fp8
            xTb = pers.tile([128, KU, N], BF16, tag="xTb")    # inputs^T bf16
            x1Tb = pers.tile([128, KU, N], BF16, tag="x1Tb")
            x2T8 = pers.tile([128, KU, N], FP8, tag="x2T8")
            w3x8 = pers.tile([128, KU, N], FP8, tag="w3x8")
            attT8 = pers.tile([128, KU, N], FP8, tag="attT8")
            xO8 = pers.tile([128, NT, U], FP8, tag="xO8")
            tW8 = pers.tile([128, KU, U], FP8, tag="tW8")     # layer0, x16
            cW8 = pers.tile([128, KU, U], FP8, tag="cW8")     # layer0, x16
            tWb = pers.tile([128, KU, U], BF16, tag="tWb")    # layer1
            cWb = pers.tile([128, KU, U], BF16, tag="cWb")    # layer1
            fW8 = pers.tile([128, 2 * KU, U], FP8, tag="fW8")  # x16
            rWb16 = pers.tile([128, KU, U], BF16, tag="rWb16")  # x16
            rW8 = pers.tile([128, KU, U], FP8, tag="rW8")       # x16
            tbsb = pers.tile([128, H, KU], F32, tag="tbsb")
            cbsb = pers.tile([128, H, KU], F32, tag="cbsb")
            awsb = pers.tile([128, 12], F32, tag="awsb")      # w1|w2|w3 cols
            aw3 = pers.tile([128, KU], F32, tag="aw3")        # 16*w3
            w1h8p = pers.tile([128, KU, 16], FP8, tag="w1h8p")  # x16, col0
            w2h8 = pers.tile([128, KU, 1], FP8, tag="w2h8")   # x16
            ab_sb = pers.tile([1, 1], F32, tag="ab_sb")
            nab_sb = pers.tile([1, 1], F32, tag="nab_sb")
            ffb16 = pers.tile([1, U], BF16, tag="ffb16")      # x16
            frb16 = pers.tile([1, U], BF16, tag="frb16")      # x16
            thr = pers.tile([1, N], BF16, tag="thr")   # exp(-(s1+ab))
            s2f = pers.tile([128, NT], F32, tag="s2f")
            ones_row = pers.tile([1, 128], BF16, tag="ones_row")
            ones216 = pers.tile([128, 2, 16], FP8, tag="ones216")
            identf = pers.tile([128, 128], F32, tag="identf")
            ident8 = pers.tile([128, 128], FP8, tag="ident8")

            nc.vector.memset(ones_row, 1.0)
            nc.vector.memset(ones216, 1.0)
            make_identity(nc, identf)
            make_identity(nc, ident8)

            # fuse-gate weight chunks dripped through phases B+C and D
            fuse_chunks = (
                [(fWv, fW8, k, k, FP8) for k in range(2 * KU)] +
                [(rWv, rWb16, k, k, BF16) for k in range(KU)] +
                [(rWv, rW8, k, k - KU, FP8) for k in range(KU, 2 * KU)])

            def emit_fuse(ci):
                wv_, wdst_, ksrc_, kdst_, dt_ = fuse_chunks[ci]
                wsf = pers.tile([128, U], F32, tag="wsf", bufs=4,
                                name=f"wsf_{ci}")
                nc.sync.dma_start(wsf, wv_[ksrc_])
                if ci % 2 == 0:
                    nc.vector.tensor_scalar_mul(wdst_[:, kdst_, :], wsf, WS)
                else:
                    nc.scalar.mul(wdst_[:, kdst_, :], wsf, WS)

            # ======== Phase A: loads, transpose, highway layer 0 ==========
            with tc.tile_pool(name="stg", bufs=8) as stg, \
                 tc.tile_pool(name="stgw", bufs=8) as stgw, \
                 tc.tile_pool(name="stgf", bufs=4) as stgf, \
                 tc.tile_pool(name="transP", bufs=2, space="PSUM") as transP, \
                 tc.tile_pool(name="hw0P", bufs=3, space="PSUM") as hw0P:
                def emit_weights(l, wi):
                    wsrc = (tW, cW)[wi]
                    wdst = ((tW8, cW8), (tWb, cWb))[l][wi]
                    wv = wsrc[l].rearrange("(k p) m -> k p m", p=128)
                    for k in range(KU):
                        ws = stgw.tile([128, U], F32, tag="ws",
                                       name=f"ws_{l}_{wi}_{k}")
                        nc.sync.dma_start(ws, wv[k])
                        if l == 0:
                            if k % 2 == 0:
                                nc.vector.tensor_scalar_mul(
                                    wdst[:, k, :], ws, WS)
                            else:
                                nc.scalar.mul(wdst[:, k, :], ws, WS)
                        else:
                            if k % 2 == 0:
                                nc.vector.tensor_copy(wdst[:, k, :], ws)
                            else:
                                nc.scalar.copy(wdst[:, k, :], ws)

                # small tensors first (layer-0 needs biases)
                nc.sync.dma_start(
                    tbsb, tb.rearrange("l (m p) -> p l m", p=128))
                nc.sync.dma_start(
                    cbsb, cb.rearrange("l (m p) -> p l m", p=128))
                nc.sync.dma_start(
                    awsb, aW.rearrange("(w m p) -> p (w m)", p=128, w=3))
                nc.vector.memset(w1h8p, 0.0)
                nc.vector.tensor_scalar_mul(w1h8p[:, :, 0], awsb[:, 0:KU],
                                            WS)
                nc.vector.tensor_scalar_mul(
                    w2h8[:, :, 0], awsb[:, KU:2 * KU], WS)
                nc.vector.tensor_scalar_mul(aw3, awsb[:, 2 * KU:3 * KU], WS)
                nc.sync.dma_start(ab_sb, ab[None, :])
                nc.scalar.mul(nab_sb, ab_sb, -1.0)
                fb = stg.tile([1, U], F32, tag="fb")
                nc.sync.dma_start(fb, ffb[None, :])
                nc.scalar.mul(ffb16, fb, WS)
                fb2 = stg.tile([1, U], F32, tag="fb")
                nc.sync.dma_start(fb2, frb[None, :])
                nc.scalar.mul(frb16, fb2, WS)

                # warm the PE HAM clock-gate during the initial DMA wait
                # warm matmuls prime hw0P's own banks (no extra psum)
                wpt = hw0P.tile([128, 512], F32, tag="pt", name="warm_pt")
                wpc = hw0P.tile([128, 512], F32, tag="pc", name="warm_pc")
                for i in range(24):
                    nc.tensor.matmul((wpt, wpc)[i % 2][:, 0:128],
                                     identf, identf,
                                     start=True, stop=True)

                def trans_block(tg, tt):
                    t = tg * 4 + tt
                    xs = stg.tile([128, U], F32, tag="xs",
                                  name=f"xs_{t}")
                    nc.sync.dma_start(xs, xv[t])
                    ptt = transP.tile([128, 512], F32, tag="ptt",
                                      name=f"ptt_{t}")
                    for k in range(KU):
                        nc.tensor.transpose(
                            ptt[:, k * 128:(k + 1) * 128],
                            xs[:, k * 128:(k + 1) * 128], identf)
                    csl = slice(tg * 512 + tt * 128,
                                tg * 512 + (tt + 1) * 128)
                    pv = ptt.rearrange("p (k c) -> p k c", k=KU)
                    nc.vector.tensor_copy(xT8[:, :, csl], pv)
                    if tt % 2 == 0:
                        nc.scalar.copy(xTb[:, :, csl], pv)
                    else:
                        nc.vector.tensor_copy(xTb[:, :, csl], pv)

                for tt in range(4):
                    trans_block(0, tt)
                emit_weights(0, 0)
                emit_weights(0, 1)
                def trans_block(tg, tt):
                    t = tg * 4 + tt
                    xs = stg.tile([128, U], F32, tag="xs",
                                  name=f"xs_{t}")
                    nc.sync.dma_start(xs, xv[t])
                    ptt = transP.tile([128, 512], F32, tag="ptt",
                                      name=f"ptt_{t}")
                    for k in range(KU):
                        nc.tensor.transpose(
                            ptt[:, k * 128:(k + 1) * 128],
                            xs[:, k * 128:(k + 1) * 128], identf)
                    csl = slice(tg * 512 + tt * 128,
                                tg * 512 + (tt + 1) * 128)
                    pv = ptt.rearrange("p (k c) -> p k c", k=KU)
                    nc.vector.tensor_copy(xT8[:, :, csl], pv)
                    if tt % 2 == 0:
                        nc.scalar.copy(xTb[:, :, csl], pv)
                    else:
                        nc.vector.tensor_copy(xTb[:, :, csl], pv)

                for tt in range(4):
                    trans_block(0, tt)
                emit_weights(0, 0)
                emit_weights(0, 1)
                for tg in range(NS):
                    nsl = slice(tg * 512, (tg + 1) * 512)
                    # highway layer 0 (fp8 DR), with tg+1's transposes
                    # interleaved between DR groups to keep HAM warm and
                    # hide the transpose latency
                    for m in range(KU):
                        pt = hw0P.tile([128, 512], F32, tag="pt")
                        pc = hw0P.tile([128, 512], F32, tag="pc")
                        for kk in range(2):
                            ksl = slice(2 * kk, 2 * kk + 2)
                            nc.tensor.matmul(
                                pt, tW8[:, ksl, m * 128:(m + 1) * 128],
                                xT8[:, ksl, nsl],
                                start=(kk == 0), stop=(kk == 1),
                                perf_mode=DR)
                        for kk in range(2):
                            ksl = slice(2 * kk, 2 * kk + 2)
                            nc.tensor.matmul(
                                pc, cW8[:, ksl, m * 128:(m + 1) * 128],
                                xT8[:, ksl, nsl],
                                start=(kk == 0), stop=(kk == 1),
                                perf_mode=DR)
                        if tg + 1 < NS:
                            trans_block(tg + 1, m)
                        th = stg.tile([128, 512], BF16, tag="th",
                                      name=f"th0_{tg}_{m}")
                        ch = stg.tile([128, 512], BF16, tag="ch",
                                      name=f"ch0_{tg}_{m}")
                        nc.scalar.activation(
                            th, pt, AF.Relu, bias=tbsb[:, 0, m:m + 1],
                            scale=IWS)
                        nc.scalar.activation(
                            ch, pc, AF.Sigmoid, bias=cbsb[:, 0, m:m + 1],
                            scale=IWS)
                        dh = stg.tile([128, 512], BF16, tag="dh",
                                      name=f"dh0_{tg}_{m}")
                        nc.vector.tensor_tensor(
                            dh, th, xTb[:, m, nsl], op=OP.subtract)
                        mh = stg.tile([128, 512], BF16, tag="mh",
                                      name=f"mh0_{tg}_{m}")
                        nc.gpsimd.tensor_tensor(mh, ch, dh, op=OP.mult)
                        nc.vector.tensor_tensor(
                            x1Tb[:, m, nsl], xTb[:, m, nsl], mh, op=OP.add)
                    # drip layer-1 weights behind the layer-0 compute
                    if tg == 0:
                        emit_weights(1, 0)
                    elif tg == 1:
                        emit_weights(1, 1)

            # ===== Phase B+C: highway layer 1 (bf16) fused with
                # attention prep (xO8 transposes, w3x, s1, s2) ============
                with tc.tile_pool(name="hwp", bufs=2, space="PSUM") as hwp, \
                     tc.tile_pool(name="xop", bufs=2, space="PSUM") as xop, \
                     tc.tile_pool(name="pc1", bufs=1, space="PSUM") as pc1, \
                     tc.tile_pool(name="hws", bufs=3) as hws:
                    ps1 = pc1.tile([16, 512], F32, tag="ps1")
                s2p = pc1.tile([128, NT], F32, tag="s2p")

                def l1_block(t):
                    nsl = slice(t * 512, (t + 1) * 512)
                    for m in range(KU):
                        pt = hwp.tile([128, 512], F32, tag="pt",
                                      name=f"pt1_{t}_{m}")
                        pc = hwp.tile([128, 512], F32, tag="pc",
                                      name=f"pc1_{t}_{m}")
                        for k in range(KU):
                            nc.tensor.matmul(
                                pt, tWb[:, k, m * 128:(m + 1) * 128],
                                x1Tb[:, k, nsl],
                                start=(k == 0), stop=(k == KU - 1))
                        for k in range(KU):
                            nc.tensor.matmul(
                                pc, cWb[:, k, m * 128:(m + 1) * 128],
                                x1Tb[:, k, nsl],
                                start=(k == 0), stop=(k == KU - 1))
                        th = hws.tile([128, 512], BF16, tag="th",
                                      name=f"th1_{t}_{m}")
                        ch = hws.tile([128, 512], BF16, tag="ch",
                                      name=f"ch1_{t}_{m}")
                        nc.scalar.activation(
                            th, pt, AF.Relu, bias=tbsb[:, 1, m:m + 1])
                        nc.scalar.activation(
                            ch, pc, AF.Sigmoid, bias=cbsb[:, 1, m:m + 1])
                        dh = hws.tile([128, 512], BF16, tag="dh",
                                      name=f"dh1_{t}_{m}")
                        nc.vector.tensor_tensor(
                            dh, th, x1Tb[:, m, nsl], op=OP.subtract)
                        mh = hws.tile([128, 512], BF16, tag="mh",
                                      name=f"mh1_{t}_{m}")
                        nc.gpsimd.tensor_tensor(mh, ch, dh, op=OP.mult)
                        nc.vector.tensor_tensor(
                            x2T8[:, m, nsl], x1Tb[:, m, nsl], mh,
                            op=OP.add)

                def prep_block(t):
                    # attention prep for a finished 512-column group;
                    # emitted one slice late so the in-order PE queue
                    # never waits on the x2 elementwise chain
                    nsl = slice(t * 512, (t + 1) * 512)
                    for k in range(KU):
                        nc.vector.tensor_scalar_mul(
                            w3x8[:, k, nsl], x2T8[:, k, nsl],
                            aw3[:, k:k + 1])
                    for jt in range(4 * t, 4 * t + 4):
                        ptr = xop.tile([128, 512, 2], FP8, tag="ptr",
                                       name=f"ptr_{jt}")
                        for k in range(KU):
                            nc.tensor.transpose(
                                ptr[:, k * 128:(k + 1) * 128, 0],
                                x2T8[:, k, jt * 128:(jt + 1) * 128],
                                ident8)
                        if jt % 2 == 0:
                            nc.vector.tensor_copy(
                                xO8[:, jt, :], ptr[:, :, 0])
                        else:
                            nc.scalar.copy(xO8[:, jt, :], ptr[:, :, 0])
                        for k in range(KU):
                            nc.tensor.matmul(
                                s2p[:, jt:jt + 1],
                                x2T8[:, k, jt * 128:(jt + 1) * 128],
                                w2h8[:, k, :],
                                start=(k == 0), stop=(k == KU - 1))
                    for kk in range(2):
                        ksl = slice(2 * kk, 2 * kk + 2)
                        nc.tensor.matmul(ps1, w1h8p[:, ksl, :],
                                         x2T8[:, ksl, nsl],
                                         start=(kk == 0), stop=(kk == 1),
                                         perf_mode=DR)
                    nc.scalar.activation(
                        thr[:, nsl], ps1[0:1, :], AF.Exp,
                        bias=nab_sb, scale=-IWS)
                    emit_fuse(2 * t)
                    emit_fuse(2 * t + 1)

                for t in range(NS):
                    l1_block(t)
                    if t > 0:
                        prep_block(t - 1)
                prep_block(NS - 1)
                nc.scalar.mul(s2f, s2p, IWS)

            # ============= Phase D: pairwise softmax attention =============
            with tc.tile_pool(name="pdn", bufs=1, space="PSUM") as pdn, \
                 tc.tile_pool(name="pds", bufs=3, space="PSUM") as pds, \
                 tc.tile_pool(name="prp", bufs=1, space="PSUM") as prp, \
                 tc.tile_pool(name="stgf", bufs=4) as stgf, \
                 tc.tile_pool(name="dsb", bufs=4) as dsb:
                def make_tail(b, h, isl, rec, pn, pnh):
                    def tail():
                        rech = dsb.tile([1, 512], BF16, tag="rech",
                                        name=f"rech_{b}_{h}")
                        nc.vector.tensor_copy(rech, rec)
                        rbc = dsb.tile([128, 512], BF16, tag="rbc",
                                       name=f"rbc_{b}_{h}")
                        pb2 = pds.tile([128, 512], F32, tag="ps",
                                       name=f"pb2_{b}_{h}")
                        nc.tensor.matmul(pb2, ones_row, rech,
                                         start=True, stop=True)
                        nc.scalar.copy(rbc, pb2)
                        for du in range(KU):
                            nc.vector.tensor_tensor(
                                attT8[:, du, isl], pnh[du], rbc,
                                op=OP.mult)
                    return tail

                deferred = None
                for b in range(BPC):
                    for h in range(IH):
                        unit = b * IH + h
                        # remaining fuse-gate weight chunks (2 per unit)
                        if unit < 4:
                            emit_fuse(8 + 2 * unit)
                            emit_fuse(8 + 2 * unit + 1)
                        isl = slice(b * L + h * 512, b * L + (h + 1) * 512)
                        pn = [pdn.tile([128, 512], F32, tag=f"pn{du}",
                                       name=f"pn_{b}_{h}_{du}")
                              for du in range(KU)]
                        pr16 = prp.tile([16, 512], F32, tag="pr16",
                                        name=f"pr16_{b}_{h}")
                        thbc = dsb.tile([128, 512], BF16, tag="thbc")
                        pb1 = pds.tile([128, 512], F32, tag="ps",
                                       name=f"pb1_{b}_{h}")
                        nc.tensor.matmul(pb1, ones_row, thr[:, isl],
                                         start=True, stop=True)
                        nc.scalar.copy(thbc, pb1)
                        for jp in range(JT // 2):
                            eh2 = dsb.tile([128, 2, 512], FP8, tag="eh2",
                                           name=f"eh2_{b}_{h}_{jp}")
                            eh2b = dsb.tile([128, 2, 512], BF16, tag="eh2b",
                                            name=f"eh2b_{b}_{h}_{jp}")
                            for g in range(2):
                                jt = 2 * jp + g
                                jg = b * JT + jt
                                jsl = slice(b * L + jt * 128,
                                            b * L + (jt + 1) * 128)
                                ps = pds.tile([128, 512], F32, tag="ps",
                                              name=f"ps_{b}_{h}_{jt}")
                                for kk in range(2):
                                    ksl = slice(2 * kk, 2 * kk + 2)
                                    nc.tensor.matmul(
                                        ps, w3x8[:, ksl, jsl],
                                        x2T8[:, ksl, isl],
                                        start=(kk == 0), stop=(kk == 1),
                                        perf_mode=DR)
                                nc.scalar.activation(
                                    eh2b[:, g, :], ps, AF.Exp,
                                    bias=s2f[:, jg:jg + 1], scale=IWS)
                                nc.vector.tensor_tensor(
                                    eh2[:, g, :], eh2b[:, g, :], thbc,
                                    op=OP.max)
                            jg0 = b * JT + 2 * jp
                            for du in range(KU):
                                nc.tensor.matmul(
                                    pn[du],
                                    xO8[:, jg0:jg0 + 2,
                                        du * 128:(du + 1) * 128],
                                    eh2,
                                    start=(jp == 0), stop=(jp == 3),
                                    perf_mode=DR)
                            nc.tensor.matmul(pr16, ones216, eh2,
                                             start=(jp == 0), stop=(jp == 3),
                                             perf_mode=DR)
                        rec = dsb.tile([1, 512], F32, tag="rec",
                                       name=f"rec_{b}_{h}")
                        nc.vector.reciprocal_approx_fast(rec, pr16[0:1, :])
                        # drain pn psum banks promptly (frees them for the
                        # next unit); the normalize tail is deferred past
                        # the next unit's matmuls so the in-order PE queue
                        # never waits on the reciprocal chain
                        pnh = [dsb.tile([128, 512], BF16, tag="pnh",
                                        bufs=8, name=f"pnh_{b}_{h}_{du}")
                               for du in range(KU)]
                        for du in range(KU):
                            if du % 2 == 0:
                                nc.scalar.copy(pnh[du], pn[du])
                            else:
                                nc.vector.tensor_copy(pnh[du], pn[du])
                        if deferred is not None:
                            deferred()
                        deferred = make_tail(b, h, isl, rec, pn, pnh)
                deferred()

            # ============= Phase E: fuse gates + output ====================
            with tc.tile_pool(name="pep", bufs=2, space="PSUM") as pep, \
                 tc.tile_pool(name="esb", bufs=3) as esb:
                for mt in range(NT):
                    msl = slice(mt * 128, (mt + 1) * 128)
                    x0t = esb.tile([128, U], F32, tag="x0t")
                    nc.sync.dma_start(x0t, xv[mt])
                    pz = pep.tile([128, 512], F32, tag="pz")
                    pr2 = pep.tile([128, 512], F32, tag="pr2")
                    for kk in range(4):
                        if kk < 2:
                            lhsT = xT8[:, 2 * kk:2 * kk + 2, msl]
                        else:
                            lhsT = attT8[:, 2 * (kk - 2):2 * (kk - 2) + 2,
                                         msl]
                        wsl = slice(2 * kk, 2 * kk + 2)
                        nc.tensor.matmul(pz, lhsT, fW8[:, wsl, :],
                                         start=(kk == 0), stop=False,
                                         perf_mode=DR)
                    for k in range(KU):
                        nc.tensor.matmul(pr2, xTb[:, k, msl], rWb16[:, k, :],
                                         start=(k == 0), stop=False)
                    for kk in range(2):
                        ksl = slice(2 * kk, 2 * kk + 2)
                        nc.tensor.matmul(pr2, attT8[:, ksl, msl],
                                         rW8[:, ksl, :],
                                         start=False, stop=False,
                                         perf_mode=DR)
                    nc.tensor.matmul(pz, ones_row, ffb16, start=False,
                                     stop=True)
                    nc.tensor.matmul(pr2, ones_row, frb16, start=False,
                                     stop=True)
                    zh = esb.tile([128, U], BF16, tag="zh")
                    rh = esb.tile([128, U], BF16, tag="rh")
                    q = esb.tile([128, U], BF16, tag="q")
                    p2 = esb.tile([128, U], F32, tag="p2")
                    ot = esb.tile([128, U], F32, tag="ot")
                    if mt == NT - 1:
                        # last unit sets the kernel tail: shorten its
                        # serial chain by splitting across engines
                        hU = U // 2
                        nc.scalar.activation(zh, pz, AF.Sigmoid, scale=IWS)
                        nc.vector.tensor_tensor(q, zh, zh, op=OP.mult)
                        nc.scalar.activation(rh, pr2, AF.Sigmoid, scale=IWS)
                        nc.vector.tensor_tensor(p2[:, :hU], rh[:, :hU],
                                                x0t[:, :hU], op=OP.mult)
                        nc.gpsimd.tensor_tensor(p2[:, hU:], rh[:, hU:],
                                                x0t[:, hU:], op=OP.mult)
                        nc.vector.tensor_tensor(ot[:, :hU], q[:, :hU],
                                                p2[:, :hU], op=OP.add)
                        nc.gpsimd.tensor_tensor(ot[:, hU:], q[:, hU:],
                                                p2[:, hU:], op=OP.add)
                    else:
                        nc.scalar.activation(zh, pz, AF.Sigmoid, scale=IWS)
                        nc.scalar.activation(rh, pr2, AF.Sigmoid, scale=IWS)
                        nc.vector.tensor_tensor(q, zh, zh, op=OP.mult)
                        nc.gpsimd.tensor_tensor(p2, rh, x0t, op=OP.mult)
                        nc.vector.tensor_tensor(ot, q, p2, op=OP.add)
                    nc.sync.dma_start(outv[mt], ot)

    nc.compile()
    return nc


_NC_CACHE = None


def _get_nc():
    global _NC_CACHE
    if _NC_CACHE is None:
        _NC_CACHE = build_nc()
    return _NC_CACHE


def kernel(**inputs) -> np.ndarray:
    from concourse.bass_utils import run_bass_kernel_spmd

    nc = _get_nc()
    full = {k: np.ascontiguousarray(np.asarray(v, dtype=np.float32))
            for k, v in inputs.items()}
    in_maps = []
    for c in range(NCORES):
        m = dict(full)
        m["inputs"] = np.ascontiguousarray(
            full["inputs"][c * BPC:(c + 1) * BPC])
        in_maps.append(m)
    res = run_bass_kernel_spmd(nc, in_maps, core_ids=list(range(NCORES)))
    return np.concatenate([res.results[c]["out"] for c in range(NCORES)],
                          axis=0)


# revision 35
# speedup vs baseline: 1.0726x; 1.0238x over previous
"""Trainium2 Bass kernel for nn_Encoding_layer (highway stack + pairwise MLP
attention + fuse gates).

Sharding: data-parallel over batch B=16 across 8 NeuronCores (2 batches per
core); all dense weights replicated. No collectives.

Mixed fp8/bf16 design:
  - Attention scores + numerator/denominator and the fuse z-gate run in
    fp8e4 with perf_mode=DoubleRow (two 128-contraction k-tiles per
    instruction).  Highway layer 0 is also fp8-DoubleRow and interleaved
    into the input-transpose phase so the PE chews matmuls while the
    remaining input tiles DMA in.  fp8 weights are scaled x16 before the
    cast (raw 0.02-scale weights sit in e4m3's subnormal range); the 1/16
    is folded into the scalar-engine activation `scale` on the psum drain.
  - Highway layer 1 and the fuse r-gate's inputs-half stay bf16: their
    error reaches the output un-smoothed (r multiplies raw inputs), while
    the fp8 parts only reach it through the near-uniform softmax over
    L=1024 (32x noise dilution) or the saturating z-gate.
  - gpsimd never touches fp8 or PSUM (ucode fallback measured ~36us/op).

Per-core layouts (n = 2 batches x L=1024 = 2048 token-columns):
  xT8 (fp8) / xTb (bf16)   : [128, 4, 2048] inputs^T  [u mod 128, u div 128, n]
  x1Tb bf16, x2T8/w3x8/attT8 fp8 : same layout
  xO8                      : [128, 16, 512] fp8 row-major highway output
  Attention: S^T[j,i] = s3[j,i] (PE, w3*x^T as lhsT) + s2[j] (ACT exp bias).
  The per-column term s1[i]+ab never enters the matmuls: a per-column
  factor cancels in the softmax, so relu becomes a clamp against
  th[i] = exp(-(s1[i]+ab)).  Numerator att^T (lhsT = row-major x,
  DoubleRow over j-tile pairs) and denominator (lhsT = fp8 ones
  [128,2,16] -> 16 identical psum rows) come from matmuls against M^T;
  normalization multiplies by the broadcast fast-approx reciprocal.
  Broadcasts of [1,512] rows are PE outer-products + scalar copies.
"""

import numpy as np

B, L, U, H = 16, 1024, 512, 2
NCORES = 8
BPC = B // NCORES          # batches per core
N = BPC * L                # token columns per core
KU = U // 128              # 4  u-tiles
NT = N // 128              # 16 row-tiles per core
NS = N // 512              # 4  512-wide column slices per core
JT = L // 128              # 8  j-tiles per batch
IH = L // 512              # 2  i-halves per batch
WS = 16.0                  # fp8 weight scale
IWS = 1.0 / WS


def build_nc():
    import concourse.bacc as bacc
    import concourse.tile as tile
    from concourse import mybir
    from concourse.masks import make_identity

    F32 = mybir.dt.float32
    BF16 = mybir.dt.bfloat16
    FP8 = mybir.dt.float8e4
    AF = mybir.ActivationFunctionType
    OP = mybir.AluOpType
    DR = mybir.MatmulPerfMode.DoubleRow

    nc = bacc.Bacc("TRN2", target_bir_lowering=False, debug=False,
                   num_devices=NCORES)

    x_in = nc.dram_tensor("inputs", [BPC, L, U], F32, kind="ExternalInput").ap()
    tW = nc.dram_tensor("tW", [H, U, U], F32, kind="ExternalInput").ap()
    tb = nc.dram_tensor("tb", [H, U], F32, kind="ExternalInput").ap()
    cW = nc.dram_tensor("cW", [H, U, U], F32, kind="ExternalInput").ap()
    cb = nc.dram_tensor("cb", [H, U], F32, kind="ExternalInput").ap()
    aW = nc.dram_tensor("aW", [3 * U], F32, kind="ExternalInput").ap()
    ab = nc.dram_tensor("ab", [1], F32, kind="ExternalInput").ap()
    frW = nc.dram_tensor("frW", [2 * U, U], F32, kind="ExternalInput").ap()
    frb = nc.dram_tensor("frb", [U], F32, kind="ExternalInput").ap()
    ffW = nc.dram_tensor("ffW", [2 * U, U], F32, kind="ExternalInput").ap()
    ffb = nc.dram_tensor("ffb", [U], F32, kind="ExternalInput").ap()
    out = nc.dram_tensor("out", [BPC, L, U], F32, kind="ExternalOutput").ap()

    xv = x_in.flatten_outer_dims().rearrange("(t p) u -> t p u", p=128)
    outv = out.flatten_outer_dims().rearrange("(t p) u -> t p u", p=128)

    fWv = ffW.rearrange("(k p) m -> k p m", p=128)
    rWv = frW.rearrange("(k p) m -> k p m", p=128)

    with tile.TileContext(nc) as tc:
        with tc.tile_pool(name="pers", bufs=1) as pers:
            # ---- persistent SBUF tensors ----
            xT8 = pers.tile([128, KU, N], FP8, tag="xT8")     # inputs^T fp8
            xTb = pers.tile([128, KU, N], BF16, tag="xTb")    # inputs^T bf16
            x1Tb = pers.tile([128, KU, N], BF16, tag="x1Tb")
            x2T8 = pers.tile([128, KU, N], FP8, tag="x2T8")
            w3x8 = pers.tile([128, KU, N], FP8, tag="w3x8")
            attT8 = pers.tile([128, KU, N], FP8, tag="attT8")
            xO8 = pers.tile([128, NT, U], FP8, tag="xO8")
            tW8 = pers.tile([128, KU, U], FP8, tag="tW8")     # layer0, x16
            cW8 = pers.tile([128, KU, U], FP8, tag="cW8")     # layer0, x16
            tWb = pers.tile([128, KU, U], BF16, tag="tWb")    # layer1
            cWb = pers.tile([128, KU, U], BF16, tag="cWb")    # layer1
            fW8 = pers.tile([128, 2 * KU, U], FP8, tag="fW8")  # x16
            rWb16 = pers.tile([128, KU, U], BF16, tag="rWb16")  # x16
            rW8 = pers.tile([128, KU, U], FP8, tag="rW8")       # x16
            tbsb = pers.tile([128, H, KU], F32, tag="tbsb")
            cbsb = pers.tile([128, H, KU], F32, tag="cbsb")
            awsb = pers.tile([128, 12], F32, tag="awsb")      # w1|w2|w3 cols
            aw3 = pers.tile([128, KU], F32, tag="aw3")        # 16*w3
            w1h8p = pers.tile([128, KU, 16], FP8, tag="w1h8p")  # x16, col0
            w2h8 = pers.tile([128, KU, 1], FP8, tag="w2h8")   # x16
            ab_sb = pers.tile([1, 1], F32, tag="ab_sb")
            nab_sb = pers.tile([1, 1], F32, tag="nab_sb")
            ffb16 = pers.tile([1, U], BF16, tag="ffb16")      # x16
            frb16 = pers.tile([1, U], BF16, tag="frb16")      # x16
            thr = pers.tile([1, N], BF16, tag="thr")   # exp(-(s1+ab))
            s2f = pers.tile([128, NT], F32, tag="s2f")
            ones_row = pers.tile([1, 128], BF16, tag="ones_row")
            ones216 = pers.tile([128, 2, 16], FP8, tag="ones216")
            identf = pers.tile([128, 128], F32, tag="identf")
            ident8 = pers.tile([128, 128], FP8, tag="ident8")

            nc.vector.memset(ones_row, 1.0)
            nc.vector.memset(ones216, 1.0)
            make_identity(nc, identf)
            make_identity(nc, ident8)

            # fuse-gate weight chunks dripped through phases B+C and D
            fuse_chunks = (
                [(fWv, fW8, k, k, FP8) for k in range(2 * KU)] +
                [(rWv, rWb16, k, k, BF16) for k in range(KU)] +
                [(rWv, rW8, k, k - KU, FP8) for k in range(KU, 2 * KU)])

            def emit_fuse(ci):
                wv_, wdst_, ksrc_, kdst_, dt_ = fuse_chunks[ci]
                wsf = pers.tile([128, U], F32, tag="wsf", bufs=4,
                                name=f"wsf_{ci}")
                nc.sync.dma_start(wsf, wv_[ksrc_])
                if ci % 2 == 0:
                    nc.vector.tensor_scalar_mul(wdst_[:, kdst_, :], wsf, WS)
                else:
                    nc.scalar.mul(wdst_[:, kdst_, :], wsf, WS)

            # ======== Phase A: loads, transpose, highway layer 0 ==========
            with tc.tile_pool(name="stg", bufs=8) as stg, \
                 tc.tile_pool(name="stgw", bufs=8) as stgw, \
                 tc.tile_pool(name="stgf", bufs=4) as stgf, \
                 tc.tile_pool(name="transP", bufs=2, space="PSUM") as transP, \
                 tc.tile_pool(name="hw0P", bufs=3, space="PSUM") as hw0P:
                def emit_weights(l, wi):
                    wsrc = (tW, cW)[wi]
                    wdst = ((tW8, cW8), (tWb, cWb))[l][wi]
                    wv = wsrc[l].rearrange("(k p) m -> k p m", p=128)
                    for k in range(KU):
                        ws = stgw.tile([128, U], F32, tag="ws",
                                       name=f"ws_{l}_{wi}_{k}")
                        nc.sync.dma_start(ws, wv[k])
                        if l == 0:
                            if k % 2 == 0:
                                nc.vector.tensor_scalar_mul(
                                    wdst[:, k, :], ws, WS)
                            else:
                                nc.scalar.mul(wdst[:, k, :], ws, WS)
                        else:
                            if k % 2 == 0:
                                nc.vector.tensor_copy(wdst[:, k, :], ws)
                            else:
                                nc.scalar.copy(wdst[:, k, :], ws)

                # small tensors first (layer-0 needs biases)
                nc.sync.dma_start(
                    tbsb, tb.rearrange("l (m p) -> p l m", p=128))
                nc.sync.dma_start(
                    cbsb, cb.rearrange("l (m p) -> p l m", p=128))
                nc.sync.dma_start(
                    awsb, aW.rearrange("(w m p) -> p (w m)", p=128, w=3))
                nc.vector.memset(w1h8p, 0.0)
                nc.vector.tensor_scalar_mul(w1h8p[:, :, 0], awsb[:, 0:KU],
                                            WS)
                nc.vector.tensor_scalar_mul(
                    w2h8[:, :, 0], awsb[:, KU:2 * KU], WS)
                nc.vector.tensor_scalar_mul(aw3, awsb[:, 2 * KU:3 * KU], WS)
                nc.sync.dma_start(ab_sb, ab[None, :])
                nc.scalar.mul(nab_sb, ab_sb, -1.0)
                fb = stg.tile([1, U], F32, tag="fb")
                nc.sync.dma_start(fb, ffb[None, :])
                nc.scalar.mul(ffb16, fb, WS)
                fb2 = stg.tile([1, U], F32, tag="fb")
                nc.sync.dma_start(fb2, frb[None, :])
                nc.scalar.mul(frb16, fb2, WS)

                # warm the PE HAM clock-gate during the initial DMA wait
                # warm matmuls prime hw0P's own banks (no extra psum)
                wpt = hw0P.tile([128, 512], F32, tag="pt", name="warm_pt")
                wpc = hw0P.tile([128, 512], F32, tag="pc", name="warm_pc")
                for i in range(24):
                    nc.tensor.matmul((wpt, wpc)[i % 2][:, 0:128],
                                     identf, identf,
                                     start=True, stop=True)

                def trans_block(tg, tt):
                    t = tg * 4 + tt
                    xs = stg.tile([128, U], F32, tag="xs",
                                  name=f"xs_{t}")
                    nc.sync.dma_start(xs, xv[t])
                    ptt = transP.tile([128, 512], F32, tag="ptt",
                                      name=f"ptt_{t}")
                    for k in range(KU):
                        nc.tensor.transpose(
                            ptt[:, k * 128:(k + 1) * 128],
                            xs[:, k * 128:(k + 1) * 128], identf)
                    csl = slice(tg * 512 + tt * 128,
                                tg * 512 + (tt + 1) * 128)
                    pv = ptt.rearrange("p (k c) -> p k c", k=KU)
                    nc.vector.tensor_copy(xT8[:, :, csl], pv)
                    if tt % 2 == 0:
                        nc.scalar.copy(xTb[:, :, csl], pv)
                    else:
                        nc.vector.tensor_copy(xTb[:, :, csl], pv)

                for tt in range(4):
                    trans_block(0, tt)
                emit_weights(0, 0)
                emit_weights(0, 1)
                def trans_block(tg, tt):
                    t = tg * 4 + tt
                    xs = stg.tile([128, U], F32, tag="xs",
                                  name=f"xs_{t}")
                    nc.sync.dma_start(xs, xv[t])
                    ptt = transP.tile([128, 512], F32, tag="ptt",
                                      name=f"ptt_{t}")
                    for k in range(KU):
                        nc.tensor.transpose(
                            ptt[:, k * 128:(k + 1) * 128],
                            xs[:, k * 128:(k + 1) * 128], identf)
                    csl = slice(tg * 512 + tt * 128,
                                tg * 512 + (tt + 1) * 128)
                    pv = ptt.rearrange("p (k c) -> p k c", k=KU)
                    nc.vector.tensor_copy(xT8[:, :, csl], pv)
                    if tt % 2 == 0:
                        nc.scalar.copy(xTb[:, :, csl], pv)
                    else:
                        nc.vector.tensor_copy(xTb[:, :, csl], pv)

                for tt in range(4):
                    trans_block(0, tt)
                emit_weights(0, 0)
                emit_weights(0, 1)
                def hw_m(l, tg, m):
                    nsl = slice(tg * 512, (tg + 1) * 512)
                    xin = xTb if l == 0 else x1Tb
                    pt = hw0P.tile([128, 512], F32, tag="pt",
                                   name=f"pt{l}_{tg}_{m}")
                    pc = hw0P.tile([128, 512], F32, tag="pc",
                                   name=f"pc{l}_{tg}_{m}")
                    if l == 0:
                        for kk in range(2):
                            ksl = slice(2 * kk, 2 * kk + 2)
                            nc.tensor.matmul(
                                pt, tW8[:, ksl, m * 128:(m + 1) * 128],
                                xT8[:, ksl, nsl],
                                start=(kk == 0), stop=(kk == 1),
                                perf_mode=DR)
                        for kk in range(2):
                            ksl = slice(2 * kk, 2 * kk + 2)
                            nc.tensor.matmul(
                                pc, cW8[:, ksl, m * 128:(m + 1) * 128],
                                xT8[:, ksl, nsl],
                                start=(kk == 0), stop=(kk == 1),
                                perf_mode=DR)
                    else:
                        for k in range(KU):
                            nc.tensor.matmul(
                                pt, tWb[:, k, m * 128:(m + 1) * 128],
                                x1Tb[:, k, nsl],
                                start=(k == 0), stop=(k == KU - 1))
                        for k in range(KU):
                            nc.tensor.matmul(
                                pc, cWb[:, k, m * 128:(m + 1) * 128],
                                x1Tb[:, k, nsl],
                                start=(k == 0), stop=(k == KU - 1))
                    th = stg.tile([128, 512], BF16, tag="th",
                                  name=f"th{l}_{tg}_{m}")
                    ch = stg.tile([128, 512], BF16, tag="ch",
                                  name=f"ch{l}_{tg}_{m}")
                    nc.scalar.activation(
                        th, pt, AF.Relu, bias=tbsb[:, l, m:m + 1],
                        scale=(IWS if l == 0 else 1.0))
                    nc.scalar.activation(
                        ch, pc, AF.Sigmoid, bias=cbsb[:, l, m:m + 1],
                        scale=(IWS if l == 0 else 1.0))
                    dh = stg.tile([128, 512], BF16, tag="dh",
                                  name=f"dh{l}_{tg}_{m}")
                    nc.vector.tensor_tensor(
                        dh, th, xin[:, m, nsl], op=OP.subtract)
                    mh = stg.tile([128, 512], BF16, tag="mh",
                                  name=f"mh{l}_{tg}_{m}")
                    nc.gpsimd.tensor_tensor(mh, ch, dh, op=OP.mult)
                    if l == 0:
                        nc.vector.tensor_tensor(
                            x1Tb[:, m, nsl], xin[:, m, nsl], mh, op=OP.add)
                    else:
                        nc.vector.tensor_tensor(
                            x2T8[:, m, nsl], xin[:, m, nsl], mh, op=OP.add)

                # 3-stream pipeline: T(s) | L0(s-1) | L1(s-2), m-granular
                for s in range(1, NS + 2):
                    for m in range(KU):
                        if s - 1 < NS:
                            hw_m(0, s - 1, m)
                        if s < NS:
                            trans_block(s, m)
                        if 0 <= s - 2:
                            hw_m(1, s - 2, m)
                    if s == 1:
                        emit_weights(1, 0)
                        emit_weights(1, 1)

            # ===== Phase B+C: highway layer 1 (bf16) fused with
                # attention prep (xO8 transposes, w3x, s1, s2) ============
                with tc.tile_pool(name="hwp", bufs=2, space="PSUM") as hwp, \
                     tc.tile_pool(name="xop", bufs=2, space="PSUM") as xop, \
                     tc.tile_pool(name="pc1", bufs=1, space="PSUM") as pc1, \
                     tc.tile_pool(name="hws", bufs=3) as hws:
                    ps1 = pc1.tile([16, 512], F32, tag="ps1")
                s2p = pc1.tile([128, NT], F32, tag="s2p")

                def prep_block(t):
                    # attention prep for a finished 512-column group;
                    # emitted one slice late so the in-order PE queue
                    # never waits on the x2 elementwise chain
                    nsl = slice(t * 512, (t + 1) * 512)
                    for k in range(KU):
                        nc.vector.tensor_scalar_mul(
                            w3x8[:, k, nsl], x2T8[:, k, nsl],
                            aw3[:, k:k + 1])
                    for jt in range(4 * t, 4 * t + 4):
                        ptr = xop.tile([128, 512, 2], FP8, tag="ptr",
                                       name=f"ptr_{jt}")
                        for k in range(KU):
                            nc.tensor.transpose(
                                ptr[:, k * 128:(k + 1) * 128, 0],
                                x2T8[:, k, jt * 128:(jt + 1) * 128],
                                ident8)
                        if jt % 2 == 0:
                            nc.vector.tensor_copy(
                                xO8[:, jt, :], ptr[:, :, 0])
                        else:
                            nc.scalar.copy(xO8[:, jt, :], ptr[:, :, 0])
                        for k in range(KU):
                            nc.tensor.matmul(
                                s2p[:, jt:jt + 1],
                                x2T8[:, k, jt * 128:(jt + 1) * 128],
                                w2h8[:, k, :],
                                start=(k == 0), stop=(k == KU - 1))
                    for kk in range(2):
                        ksl = slice(2 * kk, 2 * kk + 2)
                        nc.tensor.matmul(ps1, w1h8p[:, ksl, :],
                                         x2T8[:, ksl, nsl],
                                         start=(kk == 0), stop=(kk == 1),
                                         perf_mode=DR)
                    nc.scalar.activation(
                        thr[:, nsl], ps1[0:1, :], AF.Exp,
                        bias=nab_sb, scale=-IWS)
                    emit_fuse(2 * t)
                    emit_fuse(2 * t + 1)

                for t in range(NS):
                    prep_block(t)
                nc.scalar.mul(s2f, s2p, IWS)

            # ============= Phase D: pairwise softmax attention =============
            with tc.tile_pool(name="pdn", bufs=1, space="PSUM") as pdn, \
                 tc.tile_pool(name="pds", bufs=3, space="PSUM") as pds, \
                 tc.tile_pool(name="prp", bufs=1, space="PSUM") as prp, \
                 tc.tile_pool(name="stgf", bufs=4) as stgf, \
                 tc.tile_pool(name="dsb", bufs=4) as dsb:
                def make_tail(b, h, isl, rec, pn, pnh):
                    def tail():
                        rech = dsb.tile([1, 512], BF16, tag="rech",
                                        name=f"rech_{b}_{h}")
                        nc.vector.tensor_copy(rech, rec)
                        rbc = dsb.tile([128, 512], BF16, tag="rbc",
                                       name=f"rbc_{b}_{h}")
                        pb2 = pds.tile([128, 512], F32, tag="ps",
                                       name=f"pb2_{b}_{h}")
                        nc.tensor.matmul(pb2, ones_row, rech,
                                         start=True, stop=True)
                        nc.scalar.copy(rbc, pb2)
                        for du in range(KU):
                            nc.vector.tensor_tensor(
                                attT8[:, du, isl], pnh[du], rbc,
                                op=OP.mult)
                    return tail

                deferred = None
                for b in range(BPC):
                    for h in range(IH):
                        unit = b * IH + h
                        # remaining fuse-gate weight chunks (2 per unit)
                        if unit < 4:
                            emit_fuse(8 + 2 * unit)
                            emit_fuse(8 + 2 * unit + 1)
                        isl = slice(b * L + h * 512, b * L + (h + 1) * 512)
                        pn = [pdn.tile([128, 512], F32, tag=f"pn{du}",
                                       name=f"pn_{b}_{h}_{du}")
                              for du in range(KU)]
                        pr16 = prp.tile([16, 512], F32, tag="pr16",
                                        name=f"pr16_{b}_{h}")
                        thbc = dsb.tile([128, 512], BF16, tag="thbc")
                        pb1 = pds.tile([128, 512], F32, tag="ps",
                                       name=f"pb1_{b}_{h}")
                        nc.tensor.matmul(pb1, ones_row, thr[:, isl],
                                         start=True, stop=True)
                        nc.scalar.copy(thbc, pb1)
                        for jp in range(JT // 2):
                            eh2 = dsb.tile([128, 2, 512], FP8, tag="eh2",
                                           name=f"eh2_{b}_{h}_{jp}")
                            eh2b = dsb.tile([128, 2, 512], BF16, tag="eh2b",
                                            name=f"eh2b_{b}_{h}_{jp}")
                            for g in range(2):
                                jt = 2 * jp + g
                                jg = b * JT + jt
                                jsl = slice(b * L + jt * 128,
                                            b * L + (jt + 1) * 128)
                                ps = pds.tile([128, 512], F32, tag="ps",
                                              name=f"ps_{b}_{h}_{jt}")
                                for kk in range(2):
                                    ksl = slice(2 * kk, 2 * kk + 2)
                                    nc.tensor.matmul(
                                        ps, w3x8[:, ksl, jsl],
                                        x2T8[:, ksl, isl],
                                        start=(kk == 0), stop=(kk == 1),
                                        perf_mode=DR)
                                nc.scalar.activation(
                                    eh2b[:, g, :], ps, AF.Exp,
                                    bias=s2f[:, jg:jg + 1], scale=IWS)
                                nc.vector.tensor_tensor(
                                    eh2[:, g, :], eh2b[:, g, :], thbc,
                                    op=OP.max)
                            jg0 = b * JT + 2 * jp
                            for du in range(KU):
                                nc.tensor.matmul(
                                    pn[du],
                                    xO8[:, jg0:jg0 + 2,
                                        du * 128:(du + 1) * 128],
                                    eh2,
                                    start=(jp == 0), stop=(jp == 3),
                                    perf_mode=DR)
                            nc.tensor.matmul(pr16, ones216, eh2,
                                             start=(jp == 0), stop=(jp == 3),
                                             perf_mode=DR)
                        rec = dsb.tile([1, 512], F32, tag="rec",
                                       name=f"rec_{b}_{h}")
                        nc.vector.reciprocal_approx_fast(rec, pr16[0:1, :])
                        # drain pn psum banks promptly (frees them for the
                        # next unit); the normalize tail is deferred past
                        # the next unit's matmuls so the in-order PE queue
                        # never waits on the reciprocal chain
                        pnh = [dsb.tile([128, 512], BF16, tag="pnh",
                                        bufs=8, name=f"pnh_{b}_{h}_{du}")
                               for du in range(KU)]
                        for du in range(KU):
                            if du % 2 == 0:
                                nc.scalar.copy(pnh[du], pn[du])
                            else:
                                nc.vector.tensor_copy(pnh[du], pn[du])
                        if deferred is not None:
                            deferred()
                        deferred = make_tail(b, h, isl, rec, pn, pnh)
                deferred()

            # ============= Phase E: fuse gates + output ====================
            with tc.tile_pool(name="pep", bufs=2, space="PSUM") as pep, \
                 tc.tile_pool(name="esb", bufs=3) as esb:
                for mt in range(NT):
                    msl = slice(mt * 128, (mt + 1) * 128)
                    x0t = esb.tile([128, U], F32, tag="x0t")
                    nc.sync.dma_start(x0t, xv[mt])
                    pz = pep.tile([128, 512], F32, tag="pz")
                    pr2 = pep.tile([128, 512], F32, tag="pr2")
                    for kk in range(4):
                        if kk < 2:
                            lhsT = xT8[:, 2 * kk:2 * kk + 2, msl]
                        else:
                            lhsT = attT8[:, 2 * (kk - 2):2 * (kk - 2) + 2,
                                         msl]
                        wsl = slice(2 * kk, 2 * kk + 2)
                        nc.tensor.matmul(pz, lhsT, fW8[:, wsl, :],
                                         start=(kk == 0), stop=False,
                                         perf_mode=DR)
                    for k in range(KU):
                        nc.tensor.matmul(pr2, xTb[:, k, msl], rWb16[:, k, :],
                                         start=(k == 0), stop=False)
                    for kk in range(2):
                        ksl = slice(2 * kk, 2 * kk + 2)
                        nc.tensor.matmul(pr2, attT8[:, ksl, msl],
                                         rW8[:, ksl, :],
                                         start=False, stop=False,
                                         perf_mode=DR)
                    nc.tensor.matmul(pz, ones_row, ffb16, start=False,
                                     stop=True)
                    nc.tensor.matmul(pr2, ones_row, frb16, start=False,
                                     stop=True)
                    zh = esb.tile([128, U], BF16, tag="zh")
                    rh = esb.tile([128, U], BF16, tag="rh")
                    q = esb.tile([128, U], BF16, tag="q")
                    p2 = esb.tile([128, U], F32, tag="p2")
                    ot = esb.tile([128, U], F32, tag="ot")
                    if mt == NT - 1:
                        # last unit sets the kernel tail: shorten its
                        # serial chain by splitting across engines
                        hU = U // 2
                        nc.scalar.activation(zh, pz, AF.Sigmoid, scale=IWS)
                        nc.vector.tensor_tensor(q, zh, zh, op=OP.mult)
                        nc.scalar.activation(rh, pr2, AF.Sigmoid, scale=IWS)
                        nc.vector.tensor_tensor(p2[:, :hU], rh[:, :hU],
                                                x0t[:, :hU], op=OP.mult)
                        nc.gpsimd.tensor_tensor(p2[:, hU:], rh[:, hU:],
                                                x0t[:, hU:], op=OP.mult)
                        nc.vector.tensor_tensor(ot[:, :hU], q[:, :hU],
                                                p2[:, :hU], op=OP.add)
                        nc.gpsimd.tensor_tensor(ot[:, hU:], q[:, hU:],
                                                p2[:, hU:], op=OP.add)
                    else:
                        nc.scalar.activation(zh, pz, AF.Sigmoid, scale=IWS)
                        nc.scalar.activation(rh, pr2, AF.Sigmoid, scale=IWS)
                        nc.vector.tensor_tensor(q, zh, zh, op=OP.mult)
                        nc.gpsimd.tensor_tensor(p2, rh, x0t, op=OP.mult)
                        nc.vector.tensor_tensor(ot, q, p2, op=OP.add)
                    nc.sync.dma_start(outv[mt], ot)

    nc.compile()
    return nc


_NC_CACHE = None


def _get_nc():
    global _NC_CACHE
    if _NC_CACHE is None:
        _NC_CACHE = build_nc()
    return _NC_CACHE


def kernel(**inputs) -> np.ndarray:
    from concourse.bass_utils import run_bass_kernel_spmd

    nc = _get_nc()
    full = {k: np.ascontiguousarray(np.asarray(v, dtype=np.float32))
            for k, v in inputs.items()}
    in_maps = []
    for c in range(NCORES):
        m = dict(full)
        m["inputs"] = np.ascontiguousarray(
            full["inputs"][c * BPC:(c + 1) * BPC])
        in_maps.append(m)
    res = run_bass_kernel_spmd(nc, in_maps, core_ids=list(range(NCORES)))
    return np.concatenate([res.results[c]["out"] for c in range(NCORES)],
                          axis=0)
